# revision 1
# baseline (speedup 1.0000x reference)
"""Single-head causal attention (B=4, T=2048, C=1024) on 8 trn2 NeuronCores.

Sharding: 8 shards = (batch b in 0..3) x (query interleave h in 0..1).
Query rows are sharded as interleaved 256-row blocks (core h takes global
blocks {2*bg+h}), which balances the causal triangle across the core pair:
every core's four query blocks have causal extents {h, 2+h, 4+h, 6+h}
(x256 kv rows). One SPMD instruction stream serves all cores; all per-core
variation is data: gathered x slices and three [128,128] mask tiles
(m1d/m1f/m2d) that encode whether each kv block is this core's diagonal,
its future, or its past.

Device layout per core (S^T formulation -- scores kept as [kv, query] so
softmax denominators come from ones-matmuls on the TensorE and att@V
needs no transposes):
  phase A: k^T and V for kv global half 0 (kept in SBUF) and half 1
           (spilled to DRAM scratch, contiguous tile-major); q^T last from
           the gathered interleaved rows (reusing the x tile slots).
           DMA emission order is hand-matched to consumption order (the
           sync queue is serial at ~0.65us issue per descriptor).
  phase B: kv half 0 vs all query slots; per-kv-tile valid column ranges
           and mask positions come from static tables (LO128/MASKS);
           row-sums accumulate in PSUM; O^T += V^T A^T over exact ranges.
  phase C: reload half-1 k^T/V from scratch (overlaps B2).
  phase D: kv half 1; only query slots 2,3 participate (cols 512+),
           so this phase is half-sized -- the balance win.
  phase E: normalize by 1/rowsum, output projection with folded bias
           (b_eff = b_proj + w_proj @ b_v), DMA out y^T tile-major.

All matmuls run as float32r (TF32: 4x faster than fp32, max rel err
~4e-4 end-to-end); matmul chunks are kept >=256 wide (f32r is 4x slower
below that). Softmax skips max-subtraction (scores are O(1) here;
mathematically identical). Weights are host-packed into lhsT column-block
layout so every weight load is a single contiguous DMA. The scale 1/sqrt(C)
is folded into W_q/b_q; the V bias into the output bias.
"""

import sys

sys.path.insert(0, "/opt/trn_rl_repo")

import numpy as np

import concourse.bass as bass
import concourse.tile as tile
from concourse import mybir
from concourse.vector_clock import ScopedClock

FP = mybir.dt.float32
FPR = mybir.dt.float32r
AF = mybir.ActivationFunctionType

P = 128
C = 1024  # embed dim
H = 1024  # query rows per core
TL = 2048  # local kv length (own half first, then other half)
NT = C // P  # 8 tiles of 128
NEG = -1.0e9

# toggle: run matmuls as float32r (4x faster PE, slightly different numerics)
USE_F32R = True

_MAX_WAITS = 1


class _TC(tile.TileContext):
    """TileContext whose tail drain puts its global-clock waits on a nop
    (walrus rejects multi-wait Drain); excess waits are split by
    _split_waits() afterwards."""

    def _drain_and_barrier(self, tick_clock, wait_clock):
        nop_inst = self.nc.sync.nop(nofuse=True, hint="pre_drain_waits")
        wait_clock.add_sem_waits(
            nop_inst.ins, ScopedClock({None: tick_clock.global_clock})
        )
        self.nc.sync.drain()
        self.nc.all_engine_barrier()
        assert self.sems is not None
        popped = self.nc._tile_sem_poison_stack.pop()
        assert popped is self._sem_poison
        self.nc.clear_and_free_semaphores(list(self.sems.allocated().values()))
        self.nc.all_engine_barrier()


def _split_waits(nc, max_waits=_MAX_WAITS):
    """The walrus shipped here rejects instructions carrying more than
    `max_waits` sync waits. Move excess waits onto injected nops placed
    immediately before the instruction on the same engine (identical
    semantics: the engine's sequencer blocks on all of them either way)."""
    import copy

    template = nc.sync.nop(nofuse=True, hint="waitsplit_template").ins
    counter = [0]

    def make_nop(engine, waits):
        nop = copy.deepcopy(template)
        counter[0] += 1
        nop.name = f"I-wsplit-{counter[0]}"
        nop.engine = engine
        nop.sync_info = mybir.SyncInfo(on_wait=list(waits), on_update=[])
        return nop

    f = nc.m.functions[0]
    for bb in f.blocks:
        insts = bb.instructions
        if not any(
            i.sync_info and i.sync_info.on_wait and len(i.sync_info.on_wait) > max_waits
            for i in insts
        ):
            continue
        newlist = []
        for inst in insts:
            si = inst.sync_info
            if si and si.on_wait and len(si.on_wait) > max_waits:
                if inst.name == template.name:
                    newlist.append(inst)
                    continue
                waits = list(si.on_wait)
                del si.on_wait[max_waits:]
                rest = waits[max_waits:]
                while rest:
                    newlist.append(make_nop(inst.engine, rest[:max_waits]))
                    rest = rest[max_waits:]
            newlist.append(inst)
        bb.instructions[:] = newlist


MDT = FPR if USE_F32R else FP


def _mm(ap):
    return ap


def _chunks(lo, hi, step=512):
    """Split [lo, hi) into pieces <=step, avoiding <256-wide pieces where
    possible (float32r matmuls run 4x slower below 256 moving rows)."""
    out = []
    while lo < hi:
        rem = hi - lo
        if rem <= step:
            w = rem
        elif rem < step + 256:
            w = rem - 256  # leave a >=256 tail
        else:
            w = step
        out.append((lo, lo + w))
        lo += w
    return out


def _build_nc():
    nc = bass.Bass("TRN2", target_bir_lowering=False, debug=False)

    xTq = nc.dram_tensor("xTq", [C, H], MDT, kind="ExternalInput").ap()
    xTo = nc.dram_tensor("xTo", [C, H], MDT, kind="ExternalInput").ap()
    xTx = nc.dram_tensor("xTx", [C, H], MDT, kind="ExternalInput").ap()
    wqT = nc.dram_tensor("wqT", [C, C], MDT, kind="ExternalInput").ap()
    wkT = nc.dram_tensor("wkT", [C, C], MDT, kind="ExternalInput").ap()
    wvT = nc.dram_tensor("wvT", [4 * P, 4 * 512], MDT, kind="ExternalInput").ap()
    wpT = nc.dram_tensor("wpT", [C, C], MDT, kind="ExternalInput").ap()
    bq = nc.dram_tensor("bq", [P, NT], FP, kind="ExternalInput").ap()
    bk = nc.dram_tensor("bk", [P, NT], FP, kind="ExternalInput").ap()
    beff = nc.dram_tensor("beff", [P, NT], FP, kind="ExternalInput").ap()
    ones_in = nc.dram_tensor("ones_in", [P, P], MDT, kind="ExternalInput").ap()
    m1d_in = nc.dram_tensor("m1d_in", [P, P], FP, kind="ExternalInput").ap()
    m1f_in = nc.dram_tensor("m1f_in", [P, P], FP, kind="ExternalInput").ap()
    m2d_in = nc.dram_tensor("m2d_in", [P, P], FP, kind="ExternalInput").ap()
    # output in (o2-tile, chunk)-major layout; host reassembles
    yT = nc.dram_tensor("yT", [NT * 2 * P, 512], FP, kind="ExternalOutput").ap()
    # DRAM scratch for the other half's kT / V (tile-major, contiguous spills)
    skT = nc.dram_tensor("skT", [2 * C, 512], MDT)
    sV = nc.dram_tensor("sV", [4 * H, 256], MDT)

    with _TC(nc) as tc:
        with (
            tc.tile_pool(name="misc", bufs=1) as misc,
            tc.tile_pool(name="wstream", bufs=3) as wsp,
            tc.tile_pool(name="kqv", bufs=1) as kqv,
            tc.tile_pool(name="evac", bufs=3) as evac,
            tc.tile_pool(name="psum", bufs=6, space="PSUM") as pp,
            tc.tile_pool(name="psum_rs", bufs=1, space="PSUM") as pp_rs,
        ):
            # ---- constants / biases (DMAs emitted after critical loads) --
            ones_sb = misc.tile([P, P], MDT, tag="ones")
            m1d = misc.tile([P, P], FP, tag="m1d")
            m1f = misc.tile([P, P], FP, tag="m1f")
            m2d = misc.tile([P, P], FP, tag="m2d")
            bq_sb = misc.tile([P, NT], FP, tag="bq")
            bk_sb = misc.tile([P, NT], FP, tag="bk")
            beff_sb = misc.tile([P, NT], FP, tag="beff")

            # ---- persistent per-phase tensors ---------------------------
            kT = [kqv.tile([P, H], MDT, tag=f"kT{i}", name=f"kT{i}") for i in range(NT)]
            V = [kqv.tile([P, C], MDT, tag=f"V{i}", name=f"V{i}") for i in range(NT)]
            qT = [kqv.tile([P, H], MDT, tag=f"qT{i}", name=f"qT{i}") for i in range(NT)]

            # =============================================================
            # Phase A: projections (xh holds all of x^T, freed afterwards)
            # =============================================================
            with tc.tile_pool(name="xh", bufs=1) as xp:
                # kv-proj first from global-order x halves; q-proj last from
                # gathered interleaved rows (xq reuses the xho slots)
                xho = [
                    xp.tile([P, H], MDT, tag=f"xho{i}", name=f"xho{i}")
                    for i in range(NT)
                ]
                xhx = [
                    xp.tile([P, H], MDT, tag=f"xhx{i}", name=f"xhx{i}")
                    for i in range(NT)
                ]
                xhalf = [xho, xhx]
                wcol_pre = wsp.tile([P, C], MDT, tag="wcol", name="wcol_pre")
                nc.sync.dma_start(wcol_pre[:], wkT[0:P, :])
                nc.sync.dma_start(bk_sb[:], bk[:])
                for i in range(NT):
                    nc.sync.dma_start(xho[i][:], xTo[i * P : (i + 1) * P, :])

                VW = 256

                # k^T: out tile [o:128, t-chunk], lhsT = w-col slice
                def kproj(half, wv_pre=None):
                    for ot in range(NT):
                        if wv_pre is not None and ot in (4, 6):
                            oc = (ot - 4) // 2
                            t = wsp.tile(
                                [P, NT * VW], MDT, tag="wvoc", bufs=2,
                                name=f"wvp{half}_{oc}",
                            )
                            nc.sync.dma_start(t[:], wvT[oc * P : (oc + 1) * P, :])
                            wv_pre.append(t)
                        osl = slice(ot * P, (ot + 1) * P)
                        if half == 0 and ot == 0:
                            wcol = wcol_pre
                        else:
                            wcol = wsp.tile([P, C], MDT, tag="wcol", name=f"wk{half}_{ot}")
                            nc.sync.dma_start(wcol[:], wkT[osl, :])
                        for (cs, ce) in _chunks(0, H):
                            ps = pp.tile([P, 512], FP, tag="ps", name=f"psk{half}_{ot}_{cs}")
                            w = ce - cs
                            for ct in range(NT):
                                nc.tensor.matmul(
                                    ps[:, :w],
                                    lhsT=_mm(wcol[:, ct * P : (ct + 1) * P]),
                                    rhs=_mm(xhalf[half][ct][:, cs:ce]),
                                    start=(ct == 0),
                                    stop=(ct == NT - 1),
                                )
                            if half == 0:
                                nc.scalar.activation(
                                    kT[ot][:, cs:ce],
                                    ps[:, :w],
                                    AF.Identity,
                                    bias=bk_sb[:, ot : ot + 1],
                                )
                            else:  # spill global-half-1 kT to DRAM
                                ev = evac.tile([P, 512], MDT, tag="ev", name=f"evk{ot}_{cs}")
                                nc.scalar.activation(
                                    ev[:, :w],
                                    ps[:, :w],
                                    AF.Identity,
                                    bias=bk_sb[:, ot : ot + 1],
                                )
                                ci = cs // 512
                                nc.sync.dma_start(
                                    skT[ci * C + ot * P : ci * C + (ot + 1) * P, :w],
                                    ev[:, :w],
                                )

                # V: out tile [t:128, o-chunk], lhsT = xh col slice
                def vproj(half, pre=None):
                    for oc in range(C // VW):
                        ocs = slice(oc * VW, (oc + 1) * VW)
                        if pre is not None and oc < len(pre):
                            wvoc = pre[oc]
                        else:
                            wvoc = wsp.tile(
                                [P, NT * VW], MDT, tag="wvoc", bufs=2,
                                name=f"wv{half}_{oc}",
                            )
                            nc.sync.dma_start(wvoc[:], wvT[oc * P : (oc + 1) * P, :])
                        for tt in range(NT):
                            ps = pp.tile([P, 512], FP, tag="ps", name=f"psv{half}_{oc}_{tt}")
                            tsl = slice(tt * P, (tt + 1) * P)
                            for ct in range(NT):
                                nc.tensor.matmul(
                                    ps[:, :VW],
                                    lhsT=_mm(xhalf[half][ct][:, tsl]),
                                    rhs=_mm(wvoc[:, ct * VW : (ct + 1) * VW]),
                                    start=(ct == 0),
                                    stop=(ct == NT - 1),
                                )
                            if half == 0:
                                nc.vector.tensor_copy(V[tt][:, ocs], ps[:, :VW])
                            else:
                                ev = evac.tile([P, 512], MDT, tag="ev", name=f"evv{oc}_{tt}")
                                nc.vector.tensor_copy(ev[:, :VW], ps[:, :VW])
                                nc.sync.dma_start(
                                    sV[oc * H + tt * P : oc * H + (tt + 1) * P, :VW],
                                    ev[:, :VW],
                                )

                sc_qk = tc.nc.named_scope("A_qk")
                sc_qk.__enter__()
                kproj(0)
                sc_qk.__exit__(None, None, None)
                for i in range(NT):
                    nc.sync.dma_start(xhx[i][:], xTx[i * P : (i + 1) * P, :])
                nc.sync.dma_start(ones_sb[:], ones_in[:])
                nc.sync.dma_start(m1d[:], m1d_in[:])
                nc.sync.dma_start(m1f[:], m1f_in[:])
                nc.sync.dma_start(m2d[:], m2d_in[:])
                nc.sync.dma_start(bq_sb[:], bq[:])
                nc.sync.dma_start(beff_sb[:], beff[:])
                sc_v = tc.nc.named_scope("A_v")
                sc_v.__enter__()
                vproj(0)
                sc_v.__exit__(None, None, None)
                sc_qk = tc.nc.named_scope("A_qk2")
                sc_qk.__enter__()
                wv_pre = []
                kproj(1, wv_pre)
                sc_qk.__exit__(None, None, None)
                sc_v = tc.nc.named_scope("A_v2")
                sc_v.__enter__()
                vproj(1, wv_pre)
                sc_v.__exit__(None, None, None)

                # q^T last: xq tiles reuse the xho slots (WAR-ordered)
                sc_q = tc.nc.named_scope("A_q")
                sc_q.__enter__()
                xq = [
                    xp.tile([P, H], MDT, tag=f"xho{i}", name=f"xq{i}")
                    for i in range(NT)
                ]
                for i in range(NT):
                    nc.sync.dma_start(xq[i][:], xTq[i * P : (i + 1) * P, :])
                for ot in range(NT):
                    osl = slice(ot * P, (ot + 1) * P)
                    wcol = wsp.tile([P, C], MDT, tag="wcol")
                    nc.sync.dma_start(wcol[:], wqT[osl, :])
                    for (cs, ce) in _chunks(0, H):
                        ps = pp.tile([P, 512], FP, tag="ps")
                        w = ce - cs
                        for ct in range(NT):
                            nc.tensor.matmul(
                                ps[:, :w],
                                lhsT=_mm(wcol[:, ct * P : (ct + 1) * P]),
                                rhs=_mm(xq[ct][:, cs:ce]),
                                start=(ct == 0),
                                stop=(ct == NT - 1),
                            )
                        nc.scalar.activation(
                            qT[ot][:, cs:ce],
                            ps[:, :w],
                            AF.Identity,
                            bias=bq_sb[:, ot : ot + 1],
                        )
                sc_q.__exit__(None, None, None)

            # =============================================================
            # Phases B-E (attention): xh freed, AT/Oacc reuse its space
            # =============================================================
            with tc.tile_pool(name="attn", bufs=1) as ab:
                AT = [ab.tile([P, H], MDT, tag=f"AT{i}", name=f"AT{i}") for i in range(NT)]
                Oacc = [ab.tile([P, H], MDT, tag=f"O{i}", name=f"O{i}") for i in range(NT)]
                rs_sb = ab.tile([P, H], FP, tag="rs_sb")
                rs_ps = pp_rs.tile([P, H], FP, tag="rs")

                # Interleaved-256 balanced causal structure.
                # Query slots bg=0..3 hold global 256-row blocks g=2*bg+h.
                # Per kv 128-tile s (within a 512-col kv phase):
                #   valid query cols [LO128[s]*128 + 512*p, 1024)
                #   mask adds (m1d/m1f/m2d data tiles) at fixed positions.
                LO128 = [0, 0, 0, 1, 2, 2, 2, 3]
                MASKS = [
                    [(0, "m1d")],
                    [(0, "m1f"), (1, "m1d")],
                    [(0, "m2d"), (1, "m1f")],
                    [(1, "m2d")],
                    [(2, "m1d")],
                    [(2, "m1f"), (3, "m1d")],
                    [(2, "m2d"), (3, "m1f")],
                    [(3, "m2d")],
                ]
                MT = {"m1d": m1d, "m1f": m1f, "m2d": m2d}

                def scores_phase(pphase, first_rs, last_rs):
                    base = 512 * pphase
                    for s in range(NT):
                        lo = base + LO128[s] * P
                        chs = _chunks(lo, H)
                        # ct-outer: the chunk pair shares each kT lhsT, so the
                        # PE loads each stationary operand once, not twice
                        pss = [
                            pp.tile([P, 512], FP, tag="ps", name=f"pss{pphase}_{s}_{i}")
                            for i in range(len(chs))
                        ]
                        for ct in range(NT):
                            lhsT = _mm(kT[ct][:, s * P : (s + 1) * P])
                            for ps, (cs, ce) in zip(pss, chs):
                                nc.tensor.matmul(
                                    ps[:, : ce - cs],
                                    lhsT=lhsT,
                                    rhs=_mm(qT[ct][:, cs:ce]),
                                    start=(ct == 0),
                                    stop=(ct == NT - 1),
                                )
                        for ps, (cs, ce) in zip(pss, chs):
                            w = ce - cs
                            for off, mname in MASKS[s]:
                                a = base + off * P
                                if cs <= a < ce:
                                    nc.vector.tensor_add(
                                        ps[:, a - cs : a - cs + P],
                                        ps[:, a - cs : a - cs + P],
                                        MT[mname][:],
                                    )
                            nc.scalar.activation(AT[s][:, cs:ce], ps[:, :w], AF.Exp)
                    for s in range(NT):
                        lo = base + LO128[s] * P
                        for (cs, ce) in _chunks(lo, H):
                            nc.tensor.matmul(
                                rs_ps[:, cs:ce],
                                lhsT=_mm(ones_sb[:]),
                                rhs=_mm(AT[s][:, cs:ce]),
                                start=(first_rs and s == 0),
                                stop=(last_rs and s == NT - 1),
                            )

                def attv_phase(pphase, accumulate):
                    base = 512 * pphase
                    chs = _chunks(base, H)
                    for ot in range(NT):
                        osl = slice(ot * P, (ot + 1) * P)
                        # s-inner with one psum per chunk: each V lhsT loads once
                        pss = [
                            pp.tile([P, 512], FP, tag="ps", name=f"psav{pphase}_{ot}_{i}")
                            for i in range(len(chs))
                        ]
                        for s in range(NT):
                            lhsT = _mm(V[s][:, osl])
                            for ps, (cs, ce) in zip(pss, chs):
                                lo = max(cs, base + LO128[s] * P)
                                if lo >= ce:
                                    continue
                                smax = min(ce // P, NT)
                                nc.tensor.matmul(
                                    ps[:, lo - cs : ce - cs],
                                    lhsT=lhsT,
                                    rhs=_mm(AT[s][:, lo:ce]),
                                    start=(s == 0),
                                    stop=(s == NT - 1),
                                )
                        for ps, (cs, ce) in zip(pss, chs):
                            if accumulate:
                                nc.vector.tensor_add(
                                    Oacc[ot][:, cs:ce], Oacc[ot][:, cs:ce], ps[:]
                                )
                            else:
                                nc.vector.tensor_copy(Oacc[ot][:, cs:ce], ps[:])

                sc = tc.nc.named_scope("B1"); sc.__enter__()
                scores_phase(0, True, False)
                sc.__exit__(None, None, None)
                sc = tc.nc.named_scope("B2"); sc.__enter__()
                attv_phase(0, False)
                sc.__exit__(None, None, None)

                sc = tc.nc.named_scope("C"); sc.__enter__()
                # ---- phase C: reload kv global half 1 -------------------
                for i in range(NT):
                    for ci in range(2):
                        nc.sync.dma_start(
                            kT[i][:, ci * 512 : (ci + 1) * 512],
                            skT[ci * C + i * P : ci * C + (i + 1) * P, :],
                        )
                    for oc in range(4):
                        nc.sync.dma_start(
                            V[i][:, oc * 256 : (oc + 1) * 256],
                            sV[oc * H + i * P : oc * H + (i + 1) * P, :],
                        )
                sc.__exit__(None, None, None)

                sc = tc.nc.named_scope("D1"); sc.__enter__()
                scores_phase(1, False, True)
                sc.__exit__(None, None, None)
                sc = tc.nc.named_scope("D2"); sc.__enter__()
                attv_phase(1, True)
                sc.__exit__(None, None, None)

                # Oacc cols [0:512) got no phase-D contribution by
                # construction (query slots 0,1 never see kv half 1).

                sc = tc.nc.named_scope("E"); sc.__enter__()
                # ---- phase E: normalize + output projection -------------
                nc.vector.tensor_copy(rs_sb[:], rs_ps[:])
                nc.vector.reciprocal(rs_sb[:], rs_sb[:])
                for ot in range(NT):
                    nc.vector.tensor_mul(Oacc[ot][:], Oacc[ot][:], rs_sb[:])

                for o2 in range(NT):
                    osl = slice(o2 * P, (o2 + 1) * P)
                    wcol = wsp.tile([P, C], MDT, tag="wcol")
                    nc.sync.dma_start(wcol[:], wpT[osl, :])
                    echs = _chunks(0, H)
                    pss = [
                        pp.tile([P, 512], FP, tag="ps", name=f"pse{o2}_{i}")
                        for i in range(len(echs))
                    ]
                    for ot in range(NT):
                        lhsT = _mm(wcol[:, ot * P : (ot + 1) * P])
                        for ps, (cs, ce) in zip(pss, echs):
                            nc.tensor.matmul(
                                ps[:],
                                lhsT=lhsT,
                                rhs=_mm(Oacc[ot][:, cs:ce]),
                                start=(ot == 0),
                                stop=(ot == NT - 1),
                            )
                    for ps, (cs, ce) in zip(pss, echs):
                        ev = evac.tile([P, 512], FP, tag="evy")
                        nc.scalar.activation(
                            ev[:], ps[:], AF.Identity, bias=beff_sb[:, o2 : o2 + 1]
                        )
                        ci = cs // 512
                        nc.sync.dma_start(
                            yT[(o2 * 2 + ci) * P : (o2 * 2 + ci + 1) * P, :], ev[:]
                        )
                sc.__exit__(None, None, None)

    _split_waits(nc)
    return nc


_NC_CACHE = None


def _get_nc():
    global _NC_CACHE
    if _NC_CACHE is None:
        _NC_CACHE = _build_nc()
    return _NC_CACHE


def make_in_maps(x, w_qkv, b_qkv, w_proj, b_proj):
    """Host-side prep: shard + transpose inputs for the 8 cores."""
    x = np.asarray(x, dtype=np.float32)
    w_qkv = np.asarray(w_qkv, dtype=np.float32)
    b_qkv = np.asarray(b_qkv, dtype=np.float32)
    w_proj = np.asarray(w_proj, dtype=np.float32)
    b_proj = np.asarray(b_proj, dtype=np.float32)

    s = 1.0 / np.sqrt(np.float32(C))

    def pack_cols(w, bw=P):
        # [ot*bw + p(in-part), ct*P + o(out-within)] = w[ot*bw + o, ct*P + p]
        n_o = C // bw
        w4 = w.reshape(n_o, bw, NT, P).transpose(0, 3, 2, 1)
        return np.ascontiguousarray(w4).reshape(n_o * P, NT * bw)

    wqT = pack_cols(w_qkv[0:C] * s)
    wkT = pack_cols(w_qkv[C : 2 * C])
    wvT = pack_cols(w_qkv[2 * C : 3 * C], bw=256)
    wpT = pack_cols(w_proj)
    bq = np.ascontiguousarray((b_qkv[0:C] * s).reshape(NT, P).T)
    bk = np.ascontiguousarray(b_qkv[C : 2 * C].reshape(NT, P).T)
    bv = b_qkv[2 * C : 3 * C]
    beff = np.ascontiguousarray((b_proj + w_proj @ bv).reshape(NT, P).T)

    ones = np.ones((P, P), dtype=np.float32)
    # S^T layout: partition = kv index j, free = query index i;
    # visible (mask 0) where i >= j within a diagonal 128-block
    triu = np.triu(np.ones((P, P), dtype=np.float32))
    trilm = np.where(triu > 0, 0.0, NEG).astype(np.float32)
    zeros = np.zeros((P, P), dtype=np.float32)
    negs = np.full((P, P), NEG, dtype=np.float32)

    shared = dict(
        wqT=wqT, wkT=wkT, wvT=wvT, wpT=wpT, bq=bq, bk=bk, beff=beff,
        ones_in=ones,
    )
    in_maps = []
    for core in range(8):
        b, h = core // 2, core % 2
        xb = x[b]  # [T, C]
        # query rows: interleaved 256-blocks g = 2*bg + h
        qrows = np.concatenate(
            [xb[(2 * bg + h) * 256 : (2 * bg + h + 1) * 256] for bg in range(4)],
            axis=0,
        )
        in_maps.append(
            dict(
                shared,
                xTq=np.ascontiguousarray(qrows.T),
                xTo=np.ascontiguousarray(xb[0:H].T),
                xTx=np.ascontiguousarray(xb[H : 2 * H].T),
                # block-type masks (see device LO128/MASKS tables):
                # m1d: diagonal of a "diag(h=0)/full(h=1)" block
                # m1f: future-subtile of such a block (or past of T2)
                # m2d: diagonal of a "masked(h=0)/diag(h=1)" block
                m1d_in=trilm if h == 0 else zeros,
                m1f_in=negs if h == 0 else zeros,
                m2d_in=negs if h == 0 else trilm,
            )
        )
    return in_maps


def assemble_output(results):
    B = 4
    y = np.empty((B, 2 * H, C), dtype=np.float32)
    for core in range(8):
        b, h = core // 2, core % 2
        # yT layout [o2, ci, p, 512] -> rows are slot-major query cols
        yt = results[core]["yT"].reshape(NT, 2, P, 512)
        blk = yt.transpose(1, 3, 0, 2).reshape(H, C)  # [slot-major rows, C]
        blk4 = blk.reshape(4, 256, C)
        for bg in range(4):
            g = 2 * bg + h
            y[b, g * 256 : (g + 1) * 256, :] = blk4[bg]
    return y


def kernel(x, w_qkv, b_qkv, w_proj, b_proj):
    from concourse.bass_utils import run_bass_kernel_spmd

    nc = _get_nc()
    in_maps = make_in_maps(x, w_qkv, b_qkv, w_proj, b_proj)
    res = run_bass_kernel_spmd(nc, in_maps, list(range(8)))
    return assemble_output(res.results)



# revision 6
# speedup vs baseline: 1.2689x; 1.2689x over previous
"""Single-head causal attention (B=4, T=2048, C=1024) on 8 trn2 NeuronCores.

Sharding: 8 shards = (batch b in 0..3) x (query interleave h in 0..1); core h
takes interleaved 256-row query blocks {2*bg+h}, balancing the causal
triangle. One SPMD stream; per-core variation is data only (gathered q-rows
and three [128,128] mask tiles).

Math restructure vs the old baseline:
  - W_proj is folded into W_v host-side (Wt = w_proj @ w_v), deleting the
    output-projection phase entirely: y = (A @ (x Wt^T)) / rowsum + beff.
  - All matmuls run as fp8e4 DoubleRow (0.5 cycles/row, 256-deep contraction
    per instruction) with 3-term hi/lo error compensation: each operand v is
    split v = vh + vl (vh = e4m3(v), vl = e4m3(v - vh), both at natural
    scale) and products use vh*wh + vl*wh + vh*wl (the lo*lo term is ~eps^2
    and dropped). Effective precision ~bf16 at 0.75x the PE cost of bf16.
    Splits of x and the weights are free (host-side); k/q/V/A splits ride
    the existing PSUM-evacuation passes (Act: f32 scratch, Pool: hi cast,
    DVE: lo = scratch - hi).
  - Weights are shipped x32 (fp8-friendly range); the 1/32 un-scale rides
    the evacuation activations; 1/sqrt(C) rides the exp activation scale;
    exp carries a -ln(32) bias for fp8 headroom (cancels in softmax).
  - Everything stays in SBUF (fp8 halves footprints): no DRAM spill, no
    phase C reload, single attv accumulation chain over all 16 kv tiles.
"""

import sys

sys.path.insert(0, "/opt/trn_rl_repo")

import numpy as np
import ml_dtypes

import concourse.bass as bass
import concourse.tile as tile
from concourse import mybir
from concourse.vector_clock import ScopedClock
from bass_rust import AP as RAP

FP = mybir.dt.float32
BF = mybir.dt.bfloat16
F8 = mybir.dt.float8e4
AF = mybir.ActivationFunctionType
DR = mybir.MatmulPerfMode.DoubleRow
E4 = ml_dtypes.float8_e4m3

P = 128
C = 1024           # embed dim
NT = C // P        # 8 contraction tiles
T = 2048           # kv length per core
TK = T // P        # 16 kv tiles
H = 1024           # query cols per core
NEG = -1.0e9
ELN32 = -3.4657359  # -ln(32): exp headroom bias, cancels in softmax
S32 = 1.0 / 32.0

_MAX_WAITS = 1

# Interleaved-256 balanced causal structure (same tables as the baseline):
# query slots bg=0..3 hold global 256-row blocks g=2*bg+h. For kv tile S
# (0..15), valid query cols start at LO(S) = 512*(S//8) + LO128[S%8]*128,
# and MASKS[S%8] lists (query-128-block offset, mask tile) additions.
LO128 = [0, 0, 0, 1, 2, 2, 2, 3]
MASKS = [
    [(0, "m1d")],
    [(0, "m1f"), (1, "m1d")],
    [(0, "m2d"), (1, "m1f")],
    [(1, "m2d")],
    [(2, "m1d")],
    [(2, "m1f"), (3, "m1d")],
    [(2, "m2d"), (3, "m1f")],
    [(3, "m2d")],
]


def lo_of(S):
    return 512 * (S // 8) + LO128[S % 8] * P


class _TC(tile.TileContext):
    """TileContext whose tail drain puts its global-clock waits on a nop
    (walrus rejects multi-wait Drain); excess waits are split by
    _split_waits() afterwards."""

    def _drain_and_barrier(self, tick_clock, wait_clock):
        nop_inst = self.nc.sync.nop(nofuse=True, hint="pre_drain_waits")
        wait_clock.add_sem_waits(
            nop_inst.ins, ScopedClock({None: tick_clock.global_clock})
        )
        self.nc.sync.drain()
        self.nc.all_engine_barrier()
        assert self.sems is not None
        popped = self.nc._tile_sem_poison_stack.pop()
        assert popped is self._sem_poison
        self.nc.clear_and_free_semaphores(list(self.sems.allocated().values()))
        self.nc.all_engine_barrier()


def _split_waits(nc, max_waits=_MAX_WAITS):
    """Walrus rejects instructions carrying more than `max_waits` sync waits.
    Move excess waits onto injected nops placed immediately before the
    instruction on the same engine (identical semantics)."""
    import copy

    template = nc.sync.nop(nofuse=True, hint="waitsplit_template").ins
    counter = [0]

    def make_nop(engine, waits):
        nop = copy.deepcopy(template)
        counter[0] += 1
        nop.name = f"I-wsplit-{counter[0]}"
        nop.engine = engine
        nop.sync_info = mybir.SyncInfo(on_wait=list(waits), on_update=[])
        return nop

    f = nc.m.functions[0]
    for bb in f.blocks:
        insts = bb.instructions
        if not any(
            i.sync_info and i.sync_info.on_wait and len(i.sync_info.on_wait) > max_waits
            for i in insts
        ):
            continue
        newlist = []
        for inst in insts:
            si = inst.sync_info
            if si and si.on_wait and len(si.on_wait) > max_waits:
                if inst.name == template.name:
                    newlist.append(inst)
                    continue
                waits = list(si.on_wait)
                del si.on_wait[max_waits:]
                rest = waits[max_waits:]
                while rest:
                    newlist.append(make_nop(inst.engine, rest[:max_waits]))
                    rest = rest[max_waits:]
            newlist.append(inst)
        bb.instructions[:] = newlist


def _chunks(lo, hi, step=512):
    out = []
    while lo < hi:
        w = min(step, hi - lo)
        out.append((lo, lo + w))
        lo += w
    return out


def _pair(tl, off, stride, w):
    """[128, 2, w] AP over tile `tl` starting at column `off`, middle-dim
    stride `stride` (elements) — a DoubleRow operand covering two
    128-contraction slices."""
    a = tl[:]
    pstr, pcnt = a.ap[0]
    return RAP(a.tensor, a.offset + off, [[pstr, pcnt], [stride, 2], [1, w]])


def _build_nc():
    nc = bass.Bass("TRN2", target_bir_lowering=False, debug=False)

    # DRAM I/O.  x layouts: [p, ct*2048 + t] (hi | lo halves); xq likewise
    # with the core's gathered interleaved query rows.  Weights [p, hi|lo of
    # ot*1024 + ct*128 + o] (k/q, stationary layout) or [p, ct*1024 + ch]
    # (v-folded, moving layout), pre-scaled x32.
    xd = nc.dram_tensor("xd", [P, 2 * NT * T], F8, kind="ExternalInput").ap()
    xqd = nc.dram_tensor("xqd", [P, 2 * NT * H], F8, kind="ExternalInput").ap()
    wkd = nc.dram_tensor("wkd", [P, 2 * NT * C], F8, kind="ExternalInput").ap()
    wqd = nc.dram_tensor("wqd", [P, 2 * NT * C], F8, kind="ExternalInput").ap()
    wvd = nc.dram_tensor("wvd", [P, 2 * NT * C], F8, kind="ExternalInput").ap()
    bkd = nc.dram_tensor("bkd", [P, NT], FP, kind="ExternalInput").ap()
    bqd = nc.dram_tensor("bqd", [P, NT], FP, kind="ExternalInput").ap()
    bed = nc.dram_tensor("bed", [P, NT], FP, kind="ExternalInput").ap()
    onesd = nc.dram_tensor("onesd", [P, 2 * P], F8, kind="ExternalInput").ap()
    ones16d = nc.dram_tensor("ones16d", [P, 2 * P], F8, kind="ExternalInput").ap()
    m1dd = nc.dram_tensor("m1dd", [P, P], FP, kind="ExternalInput").ap()
    m1fd = nc.dram_tensor("m1fd", [P, P], FP, kind="ExternalInput").ap()
    m2dd = nc.dram_tensor("m2dd", [P, P], FP, kind="ExternalInput").ap()
    ebd = nc.dram_tensor("ebd", [P, 2], FP, kind="ExternalInput").ap()
    # y out, bf16, tile-major: row block (ot*2 + chunk) holds [p, 512]
    yT = nc.dram_tensor("yT", [NT * 2 * P, 512], BF, kind="ExternalOutput").ap()

    with _TC(nc) as tc:
        with (
            tc.tile_pool(name="misc", bufs=1) as misc,
            tc.tile_pool(name="wpool", bufs=3) as wp,
            tc.tile_pool(name="kqv", bufs=1) as kqv,
            tc.tile_pool(name="scr", bufs=6) as scp,
            tc.tile_pool(name="yev", bufs=2) as yep,
            tc.tile_pool(name="psum", bufs=6, space="PSUM") as pp,
            tc.tile_pool(name="psum_rs", bufs=1, space="PSUM") as pp_rs,
        ):
            ones8 = misc.tile([P, 2 * P], F8, tag="ones")
            ones16 = misc.tile([P, 2 * P], F8, tag="ones16")
            m1d = misc.tile([P, P], FP, tag="m1d")
            m1f = misc.tile([P, P], FP, tag="m1f")
            m2d = misc.tile([P, P], FP, tag="m2d")
            bk_sb = misc.tile([P, NT], FP, tag="bk")
            bq_sb = misc.tile([P, NT], FP, tag="bq")
            be_sb = misc.tile([P, NT], FP, tag="be")
            rs_sb = misc.tile([P, H], FP, tag="rs")
            eb_sb = misc.tile([P, 2], FP, tag="eb")
            MT = {"m1d": m1d, "m1f": m1f, "m2d": m2d}

            kTh = kqv.tile([P, NT * T], F8, tag="kTh", name="kTh")
            kTl = kqv.tile([P, NT * T], F8, tag="kTl", name="kTl")
            qTh = kqv.tile([P, NT * H], F8, tag="qTh", name="qTh")
            qTl = kqv.tile([P, NT * H], F8, tag="qTl", name="qTl")
            vh = kqv.tile([P, TK * C], F8, tag="vh", name="vh")
            vl = kqv.tile([P, TK * C], F8, tag="vl", name="vl")
            v16 = kqv.tile([P, TK * C], F8, tag="v16", name="v16")

            def evac(ps, w, dsth, dstl, off, bias, scale, func=AF.Identity):
                """PSUM -> f32 scratch (Act) -> hi fp8 (Pool) -> lo fp8 (DVE)."""
                sc = scp.tile([P, 512], FP, tag="scr")
                nc.scalar.activation(sc[:, :w], ps[:, :w], func, bias=bias, scale=scale)
                nc.gpsimd.tensor_copy(dsth[:, off : off + w], sc[:, :w])
                nc.vector.tensor_sub(
                    dstl[:, off : off + w], sc[:, :w], dsth[:, off : off + w]
                )

            # 3-term DoubleRow accumulation helper.  terms = [(rhs_part_off,
            # lhs_part_off), ...] as (moving, stationary) hi/lo halves.
            TERMS = ((0, 0), (1, 0), (0, 1))

            # =========================================================
            # K projection: out [c(ot), kv] — lhsT = wk, rhs = x
            # =========================================================
            with tc.tile_pool(name="xp", bufs=1) as xp:
                xhl = xp.tile([P, 2 * NT * T], F8, tag="xhl", name="xhl")
                xq = xp.tile([P, 2 * NT * H], F8, tag="xq", name="xq")

                wk_h = wp.tile([P, NT * C], F8, tag="w", name="wk_h")
                wk_l = wp.tile([P, NT * C], F8, tag="w", name="wk_l")
                # critical-path DMA order: wk hi, x chunk 0 (hi+lo), wk lo,
                # then the rest
                nc.sync.dma_start(wk_h[:], wkd[:, : NT * C])

                def xchunk(part, c0):
                    dst = RAP(
                        xhl[:].tensor,
                        xhl[:].offset + part * NT * T + c0,
                        [[xhl[:].ap[0][0], P], [T, NT], [1, 512]],
                    )
                    src = RAP(
                        xd.tensor,
                        xd.offset + part * NT * T + c0,
                        [[xd.ap[0][0], P], [T, NT], [1, 512]],
                    )
                    nc.sync.dma_start(dst, src)

                xchunk(0, 0)
                xchunk(1, 0)
                nc.sync.dma_start(wk_l[:], wkd[:, NT * C :])
                for c in range(1, 4):
                    xchunk(0, c * 512)
                    xchunk(1, c * 512)
                nc.sync.dma_start(xq[:], xqd)
                wq_h = wp.tile([P, NT * C], F8, tag="w", name="wq_h")
                nc.sync.dma_start(wq_h[:], wqd[:, : NT * C])
                wq_l = wp.tile([P, NT * C], F8, tag="w", name="wq_l")
                nc.sync.dma_start(wq_l[:], wqd[:, NT * C :])
                nc.sync.dma_start(ones8[:], onesd)
                nc.sync.dma_start(ones16[:], ones16d)
                nc.sync.dma_start(m1d[:], m1dd)
                nc.sync.dma_start(m1f[:], m1fd)
                nc.sync.dma_start(m2d[:], m2dd)
                nc.sync.dma_start(bk_sb[:], bkd)
                nc.sync.dma_start(bq_sb[:], bqd)
                nc.sync.dma_start(be_sb[:], bed)
                nc.sync.dma_start(eb_sb[:], ebd)

                sc_k = nc.named_scope("K")
                sc_k.__enter__()
                for ot in range(NT):
                    for cs, ce in _chunks(0, T):
                        w = ce - cs
                        ps = pp.tile([P, 512], FP, tag="ps", name=f"psk{ot}_{cs}")
                        n = 0
                        for rp, lp in TERMS:
                            for j in range(NT // 2):
                                nc.tensor.matmul(
                                    ps[:, :w],
                                    lhsT=_pair(
                                        wk_h if lp == 0 else wk_l,
                                        ot * C + j * 2 * P, P, P,
                                    ),
                                    rhs=_pair(
                                        xhl, rp * NT * T + j * 2 * T + cs, T, w
                                    ),
                                    start=(n == 0),
                                    stop=(n == 11),
                                    perf_mode=DR,
                                )
                                n += 1
                        evac(ps, w, kTh, kTl, ot * T + cs, bk_sb[:, ot : ot + 1], S32)
                sc_k.__exit__(None, None, None)

                sc_q = nc.named_scope("Q")
                sc_q.__enter__()
                for ot in range(NT):
                    for cs, ce in _chunks(0, H):
                        w = ce - cs
                        ps = pp.tile([P, 512], FP, tag="ps", name=f"psq{ot}_{cs}")
                        n = 0
                        for rp, lp in TERMS:
                            for j in range(NT // 2):
                                nc.tensor.matmul(
                                    ps[:, :w],
                                    lhsT=_pair(
                                        wq_h if lp == 0 else wq_l,
                                        ot * C + j * 2 * P, P, P,
                                    ),
                                    rhs=_pair(
                                        xq, rp * NT * H + j * 2 * H + cs, H, w
                                    ),
                                    start=(n == 0),
                                    stop=(n == 11),
                                    perf_mode=DR,
                                )
                                n += 1
                        evac(ps, w, qTh, qTl, ot * H + cs, bq_sb[:, ot : ot + 1], S32)
                sc_q.__exit__(None, None, None)

                wv_h = wp.tile([P, NT * C], F8, tag="w", name="wv_h")
                nc.sync.dma_start(wv_h[:], wvd[:, : NT * C])
                wv_l = wp.tile([P, NT * C], F8, tag="w", name="wv_l")
                nc.sync.dma_start(wv_l[:], wvd[:, NT * C :])

                sc_v = nc.named_scope("V")
                sc_v.__enter__()
                # folded-V projection: out [kv-rows(s), ch] — lhsT = x tile,
                # rhs = wv
                for s in range(TK):
                    for cs, ce in _chunks(0, C):
                        w = ce - cs
                        ps = pp.tile([P, 512], FP, tag="ps", name=f"psv{s}_{cs}")
                        n = 0
                        for rp, lp in TERMS:
                            for j in range(NT // 2):
                                nc.tensor.matmul(
                                    ps[:, :w],
                                    lhsT=_pair(
                                        xhl, lp * NT * T + j * 2 * T + s * P, T, P
                                    ),
                                    rhs=_pair(
                                        wv_h if rp == 0 else wv_l,
                                        j * 2 * C + cs, C, w,
                                    ),
                                    start=(n == 0),
                                    stop=(n == 11),
                                    perf_mode=DR,
                                )
                                n += 1
                        off = s * C + cs
                        sc = scp.tile([P, 512], FP, tag="scr")
                        nc.scalar.activation(
                            sc[:, :w], ps[:, :w], AF.Identity,
                            bias=eb_sb[:, 1:2], scale=S32,
                        )
                        nc.gpsimd.tensor_copy(vh[:, off : off + w], sc[:, :w])
                        nc.vector.tensor_sub(
                            vl[:, off : off + w], sc[:, :w], vh[:, off : off + w]
                        )
                        nc.scalar.activation(
                            v16[:, off : off + w], sc[:, :w], AF.Identity,
                            bias=eb_sb[:, 1:2], scale=1.0 / 16.0,
                        )
                sc_v.__exit__(None, None, None)

            # =========================================================
            # Attention: x pool freed, A tensors reuse its space
            # =========================================================
            with tc.tile_pool(name="ap", bufs=1) as apool:
                Ah = apool.tile([P, TK * H], F8, tag="Ah", name="Ah")
                Al16 = apool.tile([P, TK * H], F8, tag="Al16", name="Al16")
                rs_ps = pp_rs.tile([P, H], FP, tag="rsps")

                # zero the pair-union gap regions (read by rowsum/attv,
                # never written by scores): tiles S=3,7,11,15
                for S in (3, 7, 11, 15):
                    g0 = lo_of(S - 1)
                    g1 = lo_of(S)
                    nc.gpsimd.memset(Ah[:, S * H + g0 : S * H + g1], 0.0)
                    nc.gpsimd.memset(Al16[:, S * H + g0 : S * H + g1], 0.0)

                sc_s = nc.named_scope("S")
                sc_s.__enter__()
                for S in range(TK):
                    base = 512 * (S // 8)
                    for cs, ce in _chunks(lo_of(S), H):
                        w = ce - cs
                        ps = pp.tile([P, 512], FP, tag="ps", name=f"pss{S}_{cs}")
                        n = 0
                        for rp, lp in TERMS:
                            kt = kTh if lp == 0 else kTl
                            qt = qTh if rp == 0 else qTl
                            for j in range(NT // 2):
                                nc.tensor.matmul(
                                    ps[:, :w],
                                    lhsT=_pair(kt, j * 2 * T + S * P, T, P),
                                    rhs=_pair(qt, j * 2 * H + cs, H, w),
                                    start=(n == 0),
                                    stop=(n == 11),
                                    perf_mode=DR,
                                )
                                n += 1
                        for moff, mname in MASKS[S % 8]:
                            a = base + moff * P
                            if cs <= a < ce:
                                nc.vector.tensor_add(
                                    ps[:, a - cs : a - cs + P],
                                    ps[:, a - cs : a - cs + P],
                                    MT[mname][:],
                                )
                        off = S * H + cs
                        sc = scp.tile([P, 512], FP, tag="scr")
                        nc.scalar.activation(
                            sc[:, :w], ps[:, :w], AF.Exp,
                            bias=eb_sb[:, 0:1], scale=S32,
                        )
                        nc.gpsimd.tensor_copy(Ah[:, off : off + w], sc[:, :w])
                        r32 = scp.tile([P, 512], FP, tag="scr")
                        nc.vector.tensor_sub(
                            r32[:, :w], sc[:, :w], Ah[:, off : off + w]
                        )
                        nc.gpsimd.tensor_scalar_mul(
                            Al16[:, off : off + w], r32[:, :w], 16.0
                        )
                sc_s.__exit__(None, None, None)

                sc_r = nc.named_scope("R")
                sc_r.__enter__()
                # rowsums: ones @ (Ah | Al), DR pairs over kv tiles
                first = True
                for part, At in ((0, Ah), (1, Al16)):
                    ow = ones8 if part == 0 else ones16
                    for m in range(TK // 2):
                        lo = lo_of(2 * m)
                        for cs, ce in _chunks(lo, H):
                            w = ce - cs
                            nc.tensor.matmul(
                                rs_ps[:, cs:ce],
                                lhsT=_pair(ow, 0, P, P),
                                rhs=_pair(At, m * 2 * H + cs, H, w),
                                start=first and lo == 0,
                                stop=(part == 1 and m == TK // 2 - 1 and ce == H),
                                perf_mode=DR,
                            )
                        if lo == 0:
                            first = False
                nc.vector.reciprocal(rs_sb[:], rs_ps[:])
                sc_r.__exit__(None, None, None)

                sc_o = nc.named_scope("O")
                sc_o.__enter__()
                # attv: out [ch(ot), q] — lhsT = v, rhs = A; single
                # accumulation chain over all 16 kv tiles
                for ot in range(NT):
                    for cs, ce in _chunks(0, H):
                        ps = pp.tile([P, 512], FP, tag="ps", name=f"pso{ot}_{cs}")
                        mms = []
                        for At, vt in ((Ah, vh), (Al16, v16), (Ah, vl)):
                            for m in range(TK // 2):
                                lo = max(cs, lo_of(2 * m))
                                if lo >= ce:
                                    continue
                                mms.append(
                                    (
                                        _pair(vt, m * 2 * C + ot * P, C, P),
                                        _pair(At, m * 2 * H + lo, H, ce - lo),
                                        lo - cs,
                                        ce - lo,
                                    )
                                )
                        # widest range first so start=True covers everything
                        mms.sort(key=lambda t: t[3], reverse=True)
                        for i, (lt, rt, o0, w) in enumerate(mms):
                            nc.tensor.matmul(
                                ps[:, o0 : o0 + w],
                                lhsT=lt,
                                rhs=rt,
                                start=(i == 0),
                                stop=(i == len(mms) - 1),
                                perf_mode=DR,
                            )
                        w = ce - cs
                        ym = scp.tile([P, 512], FP, tag="scr")
                        nc.vector.tensor_mul(ym[:, :w], ps[:, :w], rs_sb[:, cs:ce])
                        ye = yep.tile([P, 512], BF, tag="ye")
                        nc.scalar.activation(
                            ye[:, :w], ym[:, :w], AF.Identity,
                            bias=be_sb[:, ot : ot + 1],
                        )
                        ci = cs // 512
                        nc.sync.dma_start(
                            yT[(ot * 2 + ci) * P : (ot * 2 + ci + 1) * P, :w],
                            ye[:, :w],
                        )
                sc_o.__exit__(None, None, None)

    _split_waits(nc)
    return nc


_NC_CACHE = None


def _get_nc():
    global _NC_CACHE
    if _NC_CACHE is None:
        _NC_CACHE = _build_nc()
    return _NC_CACHE


def _split8(a):
    """v -> (e4m3(v), e4m3(v - e4m3(v))) as fp8 arrays."""
    hi = a.astype(E4)
    lo = (a - hi.astype(np.float32)).astype(E4)
    return hi, lo


def _hl(a):
    h, l = _split8(np.ascontiguousarray(a, dtype=np.float32))
    return np.concatenate([h, l], axis=-1)


def make_in_maps(x, w_qkv, b_qkv, w_proj, b_proj):
    x = np.asarray(x, dtype=np.float32)
    w_qkv = np.asarray(w_qkv, dtype=np.float32)
    b_qkv = np.asarray(b_qkv, dtype=np.float32)
    w_proj = np.asarray(w_proj, dtype=np.float32)
    b_proj = np.asarray(b_proj, dtype=np.float32)

    wq, wk, wv = w_qkv[:C], w_qkv[C : 2 * C], w_qkv[2 * C :]
    bq, bk, bv = b_qkv[:C], b_qkv[C : 2 * C], b_qkv[2 * C :]
    wt = w_proj @ wv                       # folded V*proj weight
    beff = b_proj + w_proj @ bv

    def pack_stat(w):
        # [p, ot*1024 + ct*128 + o] = 32*w[ot*128+o, ct*128+p]
        w4 = (32.0 * w).reshape(NT, P, NT, P)       # [ot, o, ct, p]
        return w4.transpose(3, 0, 2, 1).reshape(P, NT * C)

    def pack_mov(w):
        # [p, ct*1024 + ch] = 32*w[ch, ct*128+p]
        w3 = (32.0 * w).reshape(C, NT, P)           # [ch, ct, p]
        return w3.transpose(2, 1, 0).reshape(P, NT * C)

    def pack_x(xr):
        # [p, ct*Tr + t] = xr[t, ct*128+p]
        Tr = xr.shape[0]
        x3 = xr.T.reshape(NT, P, Tr)                # [ct, p, t]
        return x3.transpose(1, 0, 2).reshape(P, NT * Tr)

    wkp = _hl(pack_stat(wk))
    wqp = _hl(pack_stat(wq))
    wvp = _hl(pack_mov(wt))
    bkp = np.ascontiguousarray(bk.reshape(NT, P).T)
    bqp = np.ascontiguousarray(bq.reshape(NT, P).T)
    bep = np.ascontiguousarray(beff.reshape(NT, P).T)

    ones = np.ones((P, 2 * P), dtype=np.float32).astype(E4)
    ones16 = np.full((P, 2 * P), 1.0 / 16.0, dtype=np.float32).astype(E4)
    triu = np.triu(np.ones((P, P), dtype=np.float32))
    trilm = np.where(triu > 0, 0.0, NEG).astype(np.float32)
    zeros = np.zeros((P, P), dtype=np.float32)
    negs = np.full((P, P), NEG, dtype=np.float32)

    shared = dict(
        wkd=wkp, wqd=wqp, wvd=wvp, bkd=bkp, bqd=bqp, bed=bep, onesd=ones,
        ones16d=ones16,
        ebd=np.concatenate(
            [np.full((P, 1), ELN32, np.float32), np.zeros((P, 1), np.float32)],
            axis=1,
        ),
    )
    in_maps = []
    for core in range(8):
        b, h = core // 2, core % 2
        xb = x[b]
        qrows = np.concatenate(
            [xb[(2 * bg + h) * 256 : (2 * bg + h + 1) * 256] for bg in range(4)],
            axis=0,
        )
        in_maps.append(
            dict(
                shared,
                xd=_hl(pack_x(xb)),
                xqd=_hl(pack_x(qrows)),
                m1dd=trilm if h == 0 else zeros,
                m1fd=negs if h == 0 else zeros,
                m2dd=negs if h == 0 else trilm,
            )
        )
    return in_maps


def assemble_output(results):
    B = 4
    y = np.empty((B, T, C), dtype=np.float32)
    for core in range(8):
        b, h = core // 2, core % 2
        yt = np.asarray(results[core]["yT"], dtype=np.float32)
        yt = yt.reshape(NT, 2, P, 512)
        full = yt.transpose(1, 3, 0, 2).reshape(H, C)   # [q-col, ch]
        for bg in range(4):
            g = 2 * bg + h
            y[b, g * 256 : (g + 1) * 256, :] = full[bg * 256 : (bg + 1) * 256]
    return y


def kernel(x, w_qkv, b_qkv, w_proj, b_proj):
    from concourse.bass_utils import run_bass_kernel_spmd

    nc = _get_nc()
    in_maps = make_in_maps(x, w_qkv, b_qkv, w_proj, b_proj)
    res = run_bass_kernel_spmd(nc, in_maps, list(range(8)))
    return assemble_output(res.results)


# revision 7
# speedup vs baseline: 1.4066x; 1.1085x over previous
"""Single-head causal attention (B=4, T=2048, C=1024) on 8 trn2 NeuronCores.

Sharding: 8 shards = (batch b in 0..3) x (query interleave h in 0..1); core h
takes interleaved 256-row query blocks {2*bg+h}, balancing the causal
triangle. One SPMD stream; per-core variation is data only (gathered q-rows
and three [128,128] mask tiles).

Math restructure vs the old baseline:
  - W_proj is folded into W_v host-side (Wt = w_proj @ w_v), deleting the
    output-projection phase entirely: y = (A @ (x Wt^T)) / rowsum + beff.
  - All matmuls run as fp8e4 DoubleRow (0.5 cycles/row, 256-deep contraction
    per instruction) with 3-term hi/lo error compensation: each operand v is
    split v = vh + vl (vh = e4m3(v), vl = e4m3(v - vh), both at natural
    scale) and products use vh*wh + vl*wh + vh*wl (the lo*lo term is ~eps^2
    and dropped). Effective precision ~bf16 at 0.75x the PE cost of bf16.
    Splits of x and the weights are free (host-side); k/q/V/A splits ride
    the existing PSUM-evacuation passes (Act: f32 scratch, Pool: hi cast,
    DVE: lo = scratch - hi).
  - Weights are shipped x32 (fp8-friendly range); the 1/32 un-scale rides
    the evacuation activations; 1/sqrt(C) rides the exp activation scale;
    exp carries a -ln(32) bias for fp8 headroom (cancels in softmax).
  - Everything stays in SBUF (fp8 halves footprints): no DRAM spill, no
    phase C reload, single attv accumulation chain over all 16 kv tiles.
"""

import sys

sys.path.insert(0, "/opt/trn_rl_repo")

import numpy as np
import ml_dtypes

import concourse.bass as bass
import concourse.tile as tile
from concourse import mybir
from concourse.vector_clock import ScopedClock
from bass_rust import AP as RAP

FP = mybir.dt.float32
BF = mybir.dt.bfloat16
F8 = mybir.dt.float8e4
AF = mybir.ActivationFunctionType
DR = mybir.MatmulPerfMode.DoubleRow
E4 = ml_dtypes.float8_e4m3

P = 128
C = 1024           # embed dim
NT = C // P        # 8 contraction tiles
T = 2048           # kv length per core
TK = T // P        # 16 kv tiles
H = 1024           # query cols per core
NEG = -1.0e9
ELN32 = -3.4657359  # -ln(32): exp headroom bias, cancels in softmax
S32 = 1.0 / 32.0

_MAX_WAITS = 1

# Interleaved-256 balanced causal structure (same tables as the baseline):
# query slots bg=0..3 hold global 256-row blocks g=2*bg+h. For kv tile S
# (0..15), valid query cols start at LO(S) = 512*(S//8) + LO128[S%8]*128,
# and MASKS[S%8] lists (query-128-block offset, mask tile) additions.
LO128 = [0, 0, 0, 1, 2, 2, 2, 3]
MASKS = [
    [(0, "m1d")],
    [(0, "m1f"), (1, "m1d")],
    [(0, "m2d"), (1, "m1f")],
    [(1, "m2d")],
    [(2, "m1d")],
    [(2, "m1f"), (3, "m1d")],
    [(2, "m2d"), (3, "m1f")],
    [(3, "m2d")],
]


def lo_of(S):
    return 512 * (S // 8) + LO128[S % 8] * P


class _TC(tile.TileContext):
    """TileContext whose tail drain puts its global-clock waits on a nop
    (walrus rejects multi-wait Drain); excess waits are split by
    _split_waits() afterwards."""

    def _drain_and_barrier(self, tick_clock, wait_clock):
        nop_inst = self.nc.sync.nop(nofuse=True, hint="pre_drain_waits")
        wait_clock.add_sem_waits(
            nop_inst.ins, ScopedClock({None: tick_clock.global_clock})
        )
        self.nc.sync.drain()
        self.nc.all_engine_barrier()
        assert self.sems is not None
        popped = self.nc._tile_sem_poison_stack.pop()
        assert popped is self._sem_poison
        self.nc.clear_and_free_semaphores(list(self.sems.allocated().values()))
        self.nc.all_engine_barrier()


def _split_waits(nc, max_waits=_MAX_WAITS):
    """Walrus rejects instructions carrying more than `max_waits` sync waits.
    Move excess waits onto injected nops placed immediately before the
    instruction on the same engine (identical semantics)."""
    import copy

    template = nc.sync.nop(nofuse=True, hint="waitsplit_template").ins
    counter = [0]

    def make_nop(engine, waits):
        nop = copy.deepcopy(template)
        counter[0] += 1
        nop.name = f"I-wsplit-{counter[0]}"
        nop.engine = engine
        nop.sync_info = mybir.SyncInfo(on_wait=list(waits), on_update=[])
        return nop

    f = nc.m.functions[0]
    for bb in f.blocks:
        insts = bb.instructions
        if not any(
            i.sync_info and i.sync_info.on_wait and len(i.sync_info.on_wait) > max_waits
            for i in insts
        ):
            continue
        newlist = []
        for inst in insts:
            si = inst.sync_info
            if si and si.on_wait and len(si.on_wait) > max_waits:
                if inst.name == template.name:
                    newlist.append(inst)
                    continue
                waits = list(si.on_wait)
                del si.on_wait[max_waits:]
                rest = waits[max_waits:]
                while rest:
                    newlist.append(make_nop(inst.engine, rest[:max_waits]))
                    rest = rest[max_waits:]
            newlist.append(inst)
        bb.instructions[:] = newlist


def _chunks(lo, hi, step=512):
    out = []
    while lo < hi:
        w = min(step, hi - lo)
        out.append((lo, lo + w))
        lo += w
    return out


def _pair(tl, off, stride, w):
    """[128, 2, w] AP over tile `tl` starting at column `off`, middle-dim
    stride `stride` (elements) — a DoubleRow operand covering two
    128-contraction slices."""
    a = tl[:]
    pstr, pcnt = a.ap[0]
    return RAP(a.tensor, a.offset + off, [[pstr, pcnt], [stride, 2], [1, w]])


def _build_nc():
    nc = bass.Bass("TRN2", target_bir_lowering=False, debug=False)

    # DRAM I/O.  x layouts: [p, ct*2048 + t] (hi | lo halves); xq likewise
    # with the core's gathered interleaved query rows.  Weights [p, hi|lo of
    # ot*1024 + ct*128 + o] (k/q, stationary layout) or [p, ct*1024 + ch]
    # (v-folded, moving layout), pre-scaled x32.
    xd = nc.dram_tensor("xd", [P, 2 * NT * T], F8, kind="ExternalInput").ap()
    xqd = nc.dram_tensor("xqd", [P, 2 * NT * H], F8, kind="ExternalInput").ap()
    wkd = nc.dram_tensor("wkd", [P, 2 * NT * C], F8, kind="ExternalInput").ap()
    wqd = nc.dram_tensor("wqd", [P, 2 * NT * C], F8, kind="ExternalInput").ap()
    wvd = nc.dram_tensor("wvd", [P, 2 * NT * C], F8, kind="ExternalInput").ap()
    bkd = nc.dram_tensor("bkd", [P, NT], FP, kind="ExternalInput").ap()
    bqd = nc.dram_tensor("bqd", [P, NT], FP, kind="ExternalInput").ap()
    bed = nc.dram_tensor("bed", [P, NT], FP, kind="ExternalInput").ap()
    onesd = nc.dram_tensor("onesd", [P, 2 * P], F8, kind="ExternalInput").ap()
    ones16d = nc.dram_tensor("ones16d", [P, 2 * P], F8, kind="ExternalInput").ap()
    m1dd = nc.dram_tensor("m1dd", [P, P], FP, kind="ExternalInput").ap()
    m1fd = nc.dram_tensor("m1fd", [P, P], FP, kind="ExternalInput").ap()
    m2dd = nc.dram_tensor("m2dd", [P, P], FP, kind="ExternalInput").ap()
    ebd = nc.dram_tensor("ebd", [P, 2], FP, kind="ExternalInput").ap()
    # y out, bf16, tile-major: row block (ot*2 + chunk) holds [p, 512]
    yT = nc.dram_tensor("yT", [NT * 2 * P, 512], BF, kind="ExternalOutput").ap()

    with _TC(nc) as tc:
        with (
            tc.tile_pool(name="misc", bufs=1) as misc,
            tc.tile_pool(name="wpool", bufs=3) as wp,
            tc.tile_pool(name="kqv", bufs=1) as kqv,
            tc.tile_pool(name="scr", bufs=6) as scp,
            tc.tile_pool(name="yev", bufs=2) as yep,
            tc.tile_pool(name="psum", bufs=6, space="PSUM") as pp,
            tc.tile_pool(name="psum_rs", bufs=1, space="PSUM") as pp_rs,
        ):
            ones8 = misc.tile([P, 2 * P], F8, tag="ones")
            ones16 = misc.tile([P, 2 * P], F8, tag="ones16")
            m1d = misc.tile([P, P], FP, tag="m1d")
            m1f = misc.tile([P, P], FP, tag="m1f")
            m2d = misc.tile([P, P], FP, tag="m2d")
            bk_sb = misc.tile([P, NT], FP, tag="bk")
            bq_sb = misc.tile([P, NT], FP, tag="bq")
            be_sb = misc.tile([P, NT], FP, tag="be")
            rs_sb = misc.tile([P, H], FP, tag="rs")
            eb_sb = misc.tile([P, 2], FP, tag="eb")
            MT = {"m1d": m1d, "m1f": m1f, "m2d": m2d}

            kTh = kqv.tile([P, NT * T], F8, tag="kTh", name="kTh")
            kTl = kqv.tile([P, NT * T], F8, tag="kTl", name="kTl")
            qTh = kqv.tile([P, NT * H], F8, tag="qTh", name="qTh")
            qTl = kqv.tile([P, NT * H], F8, tag="qTl", name="qTl")
            vh = kqv.tile([P, TK * C], F8, tag="vh", name="vh")
            vl = kqv.tile([P, TK * C], F8, tag="vl", name="vl")
            v16 = kqv.tile([P, TK * C], F8, tag="v16", name="v16")

            def evac(ps, w, dsth, dstl, off, bias, scale, func=AF.Identity):
                """PSUM -> f32 scratch (Act) -> hi fp8 (Pool) -> lo fp8 (DVE)."""
                sc = scp.tile([P, 512], FP, tag="scr")
                nc.scalar.activation(sc[:, :w], ps[:, :w], func, bias=bias, scale=scale)
                nc.gpsimd.tensor_copy(dsth[:, off : off + w], sc[:, :w])
                nc.vector.tensor_sub(
                    dstl[:, off : off + w], sc[:, :w], dsth[:, off : off + w]
                )

            # 3-term DoubleRow accumulation helper.  terms = [(rhs_part_off,
            # lhs_part_off), ...] as (moving, stationary) hi/lo halves.
            TERMS = ((0, 0), (1, 0), (0, 1))

            # =========================================================
            # K projection: out [c(ot), kv] — lhsT = wk, rhs = x
            # =========================================================
            with tc.tile_pool(name="xp", bufs=1) as xp:
                xhl = xp.tile([P, 2 * NT * T], F8, tag="xhl", name="xhl")
                xq = xp.tile([P, 2 * NT * H], F8, tag="xq", name="xq")

                wk_h = wp.tile([P, NT * C], F8, tag="w", name="wk_h")
                wk_l = wp.tile([P, NT * C], F8, tag="w", name="wk_l")
                # critical-path DMA order: wk slice 0 (hi+lo), x chunk 0,
                # remaining wk slices, remaining x chunks, then the rest
                nc.sync.dma_start(wk_h[:, :C], wkd[:, :C])
                nc.sync.dma_start(wk_l[:, :C], wkd[:, NT * C : NT * C + C])

                def xchunk(part, c0):
                    dst = RAP(
                        xhl[:].tensor,
                        xhl[:].offset + part * NT * T + c0,
                        [[xhl[:].ap[0][0], P], [T, NT], [1, 512]],
                    )
                    src = RAP(
                        xd.tensor,
                        xd.offset + part * NT * T + c0,
                        [[xd.ap[0][0], P], [T, NT], [1, 512]],
                    )
                    nc.sync.dma_start(dst, src)

                xchunk(0, 0)
                xchunk(1, 0)
                nc.sync.dma_start(bk_sb[:], bkd)
                for ot in range(1, NT):
                    nc.sync.dma_start(wk_h[:, ot * C : ot * C + C],
                                      wkd[:, ot * C : ot * C + C])
                    nc.sync.dma_start(wk_l[:, ot * C : ot * C + C],
                                      wkd[:, (NT + ot) * C : (NT + ot) * C + C])
                for c in range(1, 4):
                    xchunk(0, c * 512)
                    xchunk(1, c * 512)
                nc.sync.dma_start(xq[:], xqd)
                wq_h = wp.tile([P, NT * C], F8, tag="w", name="wq_h")
                nc.sync.dma_start(wq_h[:], wqd[:, : NT * C])
                wq_l = wp.tile([P, NT * C], F8, tag="w", name="wq_l")
                nc.sync.dma_start(wq_l[:], wqd[:, NT * C :])
                nc.sync.dma_start(bq_sb[:], bqd)
                nc.sync.dma_start(eb_sb[:], ebd)

                sc_k = nc.named_scope("K")
                sc_k.__enter__()
                for cs, ce in _chunks(0, T):
                    for ot in range(NT):
                        w = ce - cs
                        ps = pp.tile([P, 512], FP, tag="ps", name=f"psk{ot}_{cs}")
                        n = 0
                        for rp, lp in TERMS:
                            for j in range(NT // 2):
                                nc.tensor.matmul(
                                    ps[:, :w],
                                    lhsT=_pair(
                                        wk_h if lp == 0 else wk_l,
                                        ot * C + j * 2 * P, P, P,
                                    ),
                                    rhs=_pair(
                                        xhl, rp * NT * T + j * 2 * T + cs, T, w
                                    ),
                                    start=(n == 0),
                                    stop=(n == 11),
                                    perf_mode=DR,
                                )
                                n += 1
                        evac(ps, w, kTh, kTl, ot * T + cs, bk_sb[:, ot : ot + 1], S32)
                sc_k.__exit__(None, None, None)

                sc_q = nc.named_scope("Q")
                sc_q.__enter__()
                for ot in range(NT):
                    for cs, ce in _chunks(0, H):
                        w = ce - cs
                        ps = pp.tile([P, 512], FP, tag="ps", name=f"psq{ot}_{cs}")
                        n = 0
                        for rp, lp in TERMS:
                            for j in range(NT // 2):
                                nc.tensor.matmul(
                                    ps[:, :w],
                                    lhsT=_pair(
                                        wq_h if lp == 0 else wq_l,
                                        ot * C + j * 2 * P, P, P,
                                    ),
                                    rhs=_pair(
                                        xq, rp * NT * H + j * 2 * H + cs, H, w
                                    ),
                                    start=(n == 0),
                                    stop=(n == 11),
                                    perf_mode=DR,
                                )
                                n += 1
                        evac(ps, w, qTh, qTl, ot * H + cs, bq_sb[:, ot : ot + 1], S32)
                sc_q.__exit__(None, None, None)

                wv_h = wp.tile([P, NT * C], F8, tag="w", name="wv_h")
                nc.sync.dma_start(wv_h[:], wvd[:, : NT * C])
                wv_l = wp.tile([P, NT * C], F8, tag="w", name="wv_l")
                nc.sync.dma_start(wv_l[:], wvd[:, NT * C :])
                nc.sync.dma_start(ones8[:], onesd)
                nc.sync.dma_start(ones16[:], ones16d)
                nc.sync.dma_start(m1d[:], m1dd)
                nc.sync.dma_start(m1f[:], m1fd)
                nc.sync.dma_start(m2d[:], m2dd)
                nc.sync.dma_start(be_sb[:], bed)

                sc_v = nc.named_scope("V")
                sc_v.__enter__()
                # folded-V projection: out [kv-rows(s), ch] — lhsT = x tile,
                # rhs = wv
                for s in range(TK):
                    for cs, ce in _chunks(0, C):
                        w = ce - cs
                        ps = pp.tile([P, 512], FP, tag="ps", name=f"psv{s}_{cs}")
                        n = 0
                        for rp, lp in TERMS:
                            for j in range(NT // 2):
                                nc.tensor.matmul(
                                    ps[:, :w],
                                    lhsT=_pair(
                                        xhl, lp * NT * T + j * 2 * T + s * P, T, P
                                    ),
                                    rhs=_pair(
                                        wv_h if rp == 0 else wv_l,
                                        j * 2 * C + cs, C, w,
                                    ),
                                    start=(n == 0),
                                    stop=(n == 11),
                                    perf_mode=DR,
                                )
                                n += 1
                        off = s * C + cs
                        sc = scp.tile([P, 512], FP, tag="scr")
                        nc.scalar.activation(
                            sc[:, :w], ps[:, :w], AF.Identity,
                            bias=eb_sb[:, 1:2], scale=S32,
                        )
                        nc.gpsimd.tensor_copy(vh[:, off : off + w], sc[:, :w])
                        nc.vector.tensor_sub(
                            vl[:, off : off + w], sc[:, :w], vh[:, off : off + w]
                        )
                        nc.scalar.activation(
                            v16[:, off : off + w], sc[:, :w], AF.Identity,
                            bias=eb_sb[:, 1:2], scale=1.0 / 16.0,
                        )
                sc_v.__exit__(None, None, None)

            # =========================================================
            # Attention: x pool freed, A tensors reuse its space
            # =========================================================
            with tc.tile_pool(name="ap", bufs=1) as apool:
                Ah = apool.tile([P, TK * H], F8, tag="Ah", name="Ah")
                Al16 = apool.tile([P, TK * H], F8, tag="Al16", name="Al16")
                rs_ps = pp_rs.tile([P, H], FP, tag="rsps")

                # zero the pair-union gap regions (read by rowsum/attv,
                # never written by scores): tiles S=3,7,11,15
                for S in (3, 7, 11, 15):
                    g0 = lo_of(S - 1)
                    g1 = lo_of(S)
                    nc.gpsimd.memset(Ah[:, S * H + g0 : S * H + g1], 0.0)
                    nc.gpsimd.memset(Al16[:, S * H + g0 : S * H + g1], 0.0)

                sc_s = nc.named_scope("S")
                sc_s.__enter__()
                for S in range(TK):
                    base = 512 * (S // 8)
                    for cs, ce in _chunks(lo_of(S), H):
                        w = ce - cs
                        ps = pp.tile([P, 512], FP, tag="ps", name=f"pss{S}_{cs}")
                        n = 0
                        for rp, lp in TERMS:
                            kt = kTh if lp == 0 else kTl
                            qt = qTh if rp == 0 else qTl
                            for j in range(NT // 2):
                                nc.tensor.matmul(
                                    ps[:, :w],
                                    lhsT=_pair(kt, j * 2 * T + S * P, T, P),
                                    rhs=_pair(qt, j * 2 * H + cs, H, w),
                                    start=(n == 0),
                                    stop=(n == 11),
                                    perf_mode=DR,
                                )
                                n += 1
                        for moff, mname in MASKS[S % 8]:
                            a = base + moff * P
                            if cs <= a < ce:
                                nc.vector.tensor_add(
                                    ps[:, a - cs : a - cs + P],
                                    ps[:, a - cs : a - cs + P],
                                    MT[mname][:],
                                )
                        off = S * H + cs
                        sc = scp.tile([P, 512], FP, tag="scr")
                        nc.scalar.activation(
                            sc[:, :w], ps[:, :w], AF.Exp,
                            bias=eb_sb[:, 0:1], scale=S32,
                        )
                        nc.gpsimd.tensor_copy(Ah[:, off : off + w], sc[:, :w])
                        r32 = scp.tile([P, 512], FP, tag="scr")
                        nc.vector.tensor_sub(
                            r32[:, :w], sc[:, :w], Ah[:, off : off + w]
                        )
                        nc.scalar.activation(
                            Al16[:, off : off + w], r32[:, :w], AF.Identity,
                            bias=eb_sb[:, 1:2], scale=16.0,
                        )
                sc_s.__exit__(None, None, None)

                sc_r = nc.named_scope("R")
                sc_r.__enter__()
                # rowsums: ones @ (Ah | Al), DR pairs over kv tiles
                first = True
                for part, At in ((0, Ah), (1, Al16)):
                    ow = ones8 if part == 0 else ones16
                    for m in range(TK // 2):
                        lo = lo_of(2 * m)
                        for cs, ce in _chunks(lo, H):
                            w = ce - cs
                            nc.tensor.matmul(
                                rs_ps[:, cs:ce],
                                lhsT=_pair(ow, 0, P, P),
                                rhs=_pair(At, m * 2 * H + cs, H, w),
                                start=first and lo == 0,
                                stop=(part == 1 and m == TK // 2 - 1 and ce == H),
                                perf_mode=DR,
                            )
                        if lo == 0:
                            first = False
                nc.vector.reciprocal(rs_sb[:], rs_ps[:])
                sc_r.__exit__(None, None, None)

                sc_o = nc.named_scope("O")
                sc_o.__enter__()
                # attv: out [ch(ot), q] — lhsT = v, rhs = A; single
                # accumulation chain over all 16 kv tiles
                for ot in range(NT):
                    for cs, ce in _chunks(0, H):
                        ps = pp.tile([P, 512], FP, tag="ps", name=f"pso{ot}_{cs}")
                        mms = []
                        for At, vt in ((Ah, vh), (Al16, v16), (Ah, vl)):
                            for m in range(TK // 2):
                                lo = max(cs, lo_of(2 * m))
                                if lo >= ce:
                                    continue
                                mms.append(
                                    (
                                        _pair(vt, m * 2 * C + ot * P, C, P),
                                        _pair(At, m * 2 * H + lo, H, ce - lo),
                                        lo - cs,
                                        ce - lo,
                                    )
                                )
                        # widest range first so start=True covers everything
                        mms.sort(key=lambda t: t[3], reverse=True)
                        for i, (lt, rt, o0, w) in enumerate(mms):
                            nc.tensor.matmul(
                                ps[:, o0 : o0 + w],
                                lhsT=lt,
                                rhs=rt,
                                start=(i == 0),
                                stop=(i == len(mms) - 1),
                                perf_mode=DR,
                            )
                        w = ce - cs
                        ym = scp.tile([P, 512], FP, tag="scr")
                        nc.vector.tensor_mul(ym[:, :w], ps[:, :w], rs_sb[:, cs:ce])
                        ye = yep.tile([P, 512], BF, tag="ye")
                        nc.scalar.activation(
                            ye[:, :w], ym[:, :w], AF.Identity,
                            bias=be_sb[:, ot : ot + 1],
                        )
                        ci = cs // 512
                        nc.gpsimd.dma_start(
                            yT[(ot * 2 + ci) * P : (ot * 2 + ci + 1) * P, :w],
                            ye[:, :w],
                        )
                sc_o.__exit__(None, None, None)

    _split_waits(nc)
    return nc


_NC_CACHE = None


def _get_nc():
    global _NC_CACHE
    if _NC_CACHE is None:
        _NC_CACHE = _build_nc()
    return _NC_CACHE


def _split8(a):
    """v -> (e4m3(v), e4m3(v - e4m3(v))) as fp8 arrays."""
    hi = a.astype(E4)
    lo = (a - hi.astype(np.float32)).astype(E4)
    return hi, lo


def _hl(a):
    h, l = _split8(np.ascontiguousarray(a, dtype=np.float32))
    return np.concatenate([h, l], axis=-1)


def make_in_maps(x, w_qkv, b_qkv, w_proj, b_proj):
    x = np.asarray(x, dtype=np.float32)
    w_qkv = np.asarray(w_qkv, dtype=np.float32)
    b_qkv = np.asarray(b_qkv, dtype=np.float32)
    w_proj = np.asarray(w_proj, dtype=np.float32)
    b_proj = np.asarray(b_proj, dtype=np.float32)

    wq, wk, wv = w_qkv[:C], w_qkv[C : 2 * C], w_qkv[2 * C :]
    bq, bk, bv = b_qkv[:C], b_qkv[C : 2 * C], b_qkv[2 * C :]
    wt = w_proj @ wv                       # folded V*proj weight
    beff = b_proj + w_proj @ bv

    def pack_stat(w):
        # [p, ot*1024 + ct*128 + o] = 32*w[ot*128+o, ct*128+p]
        w4 = (32.0 * w).reshape(NT, P, NT, P)       # [ot, o, ct, p]
        return w4.transpose(3, 0, 2, 1).reshape(P, NT * C)

    def pack_mov(w):
        # [p, ct*1024 + ch] = 32*w[ch, ct*128+p]
        w3 = (32.0 * w).reshape(C, NT, P)           # [ch, ct, p]
        return w3.transpose(2, 1, 0).reshape(P, NT * C)

    def pack_x(xr):
        # [p, ct*Tr + t] = xr[t, ct*128+p]
        Tr = xr.shape[0]
        x3 = xr.T.reshape(NT, P, Tr)                # [ct, p, t]
        return x3.transpose(1, 0, 2).reshape(P, NT * Tr)

    wkp = _hl(pack_stat(wk))
    wqp = _hl(pack_stat(wq))
    wvp = _hl(pack_mov(wt))
    bkp = np.ascontiguousarray(bk.reshape(NT, P).T)
    bqp = np.ascontiguousarray(bq.reshape(NT, P).T)
    bep = np.ascontiguousarray(beff.reshape(NT, P).T)

    ones = np.ones((P, 2 * P), dtype=np.float32).astype(E4)
    ones16 = np.full((P, 2 * P), 1.0 / 16.0, dtype=np.float32).astype(E4)
    triu = np.triu(np.ones((P, P), dtype=np.float32))
    trilm = np.where(triu > 0, 0.0, NEG).astype(np.float32)
    zeros = np.zeros((P, P), dtype=np.float32)
    negs = np.full((P, P), NEG, dtype=np.float32)

    shared = dict(
        wkd=wkp, wqd=wqp, wvd=wvp, bkd=bkp, bqd=bqp, bed=bep, onesd=ones,
        ones16d=ones16,
        ebd=np.concatenate(
            [np.full((P, 1), ELN32, np.float32), np.zeros((P, 1), np.float32)],
            axis=1,
        ),
    )
    in_maps = []
    for core in range(8):
        b, h = core // 2, core % 2
        xb = x[b]
        qrows = np.concatenate(
            [xb[(2 * bg + h) * 256 : (2 * bg + h + 1) * 256] for bg in range(4)],
            axis=0,
        )
        in_maps.append(
            dict(
                shared,
                xd=_hl(pack_x(xb)),
                xqd=_hl(pack_x(qrows)),
                m1dd=trilm if h == 0 else zeros,
                m1fd=negs if h == 0 else zeros,
                m2dd=negs if h == 0 else trilm,
            )
        )
    return in_maps


def assemble_output(results):
    B = 4
    y = np.empty((B, T, C), dtype=np.float32)
    for core in range(8):
        b, h = core // 2, core % 2
        yt = np.asarray(results[core]["yT"], dtype=np.float32)
        yt = yt.reshape(NT, 2, P, 512)
        full = yt.transpose(1, 3, 0, 2).reshape(H, C)   # [q-col, ch]
        for bg in range(4):
            g = 2 * bg + h
            y[b, g * 256 : (g + 1) * 256, :] = full[bg * 256 : (bg + 1) * 256]
    return y


def kernel(x, w_qkv, b_qkv, w_proj, b_proj):
    from concourse.bass_utils import run_bass_kernel_spmd

    nc = _get_nc()
    in_maps = make_in_maps(x, w_qkv, b_qkv, w_proj, b_proj)
    res = run_bass_kernel_spmd(nc, in_maps, list(range(8)))
    return assemble_output(res.results)


# revision 8
# speedup vs baseline: 1.4410x; 1.0245x over previous
"""Single-head causal attention (B=4, T=2048, C=1024) on 8 trn2 NeuronCores.

Sharding: 8 shards = (batch b in 0..3) x (query interleave h in 0..1); core h
takes interleaved 256-row query blocks {2*bg+h}, balancing the causal
triangle. One SPMD stream; per-core variation is data only (gathered q-rows
and three [128,128] mask tiles).

Math restructure vs the old baseline:
  - W_proj is folded into W_v host-side (Wt = w_proj @ w_v), deleting the
    output-projection phase entirely: y = (A @ (x Wt^T)) / rowsum + beff.
  - All matmuls run as fp8e4 DoubleRow (0.5 cycles/row, 256-deep contraction
    per instruction) with 3-term hi/lo error compensation: each operand v is
    split v = vh + vl (vh = e4m3(v), vl = e4m3(v - vh), both at natural
    scale) and products use vh*wh + vl*wh + vh*wl (the lo*lo term is ~eps^2
    and dropped). Effective precision ~bf16 at 0.75x the PE cost of bf16.
    Splits of x and the weights are free (host-side); k/q/V/A splits ride
    the existing PSUM-evacuation passes (Act: f32 scratch, Pool: hi cast,
    DVE: lo = scratch - hi).
  - Weights are shipped x32 (fp8-friendly range); the 1/32 un-scale rides
    the evacuation activations; 1/sqrt(C) rides the exp activation scale;
    exp carries a -ln(32) bias for fp8 headroom (cancels in softmax).
  - Everything stays in SBUF (fp8 halves footprints): no DRAM spill, no
    phase C reload, single attv accumulation chain over all 16 kv tiles.
"""

import sys

sys.path.insert(0, "/opt/trn_rl_repo")

import numpy as np
import ml_dtypes

import concourse.bass as bass
import concourse.tile as tile
from concourse import mybir
from concourse.vector_clock import ScopedClock
from bass_rust import AP as RAP

FP = mybir.dt.float32
BF = mybir.dt.bfloat16
F8 = mybir.dt.float8e4
AF = mybir.ActivationFunctionType
DR = mybir.MatmulPerfMode.DoubleRow
E4 = ml_dtypes.float8_e4m3

P = 128
C = 1024           # embed dim
NT = C // P        # 8 contraction tiles
T = 2048           # kv length per core
TK = T // P        # 16 kv tiles
H = 1024           # query cols per core
NEG = -1.0e9
ELN32 = -3.4657359  # -ln(32): exp headroom bias, cancels in softmax
S32 = 1.0 / 32.0

_MAX_WAITS = 1

# Interleaved-256 balanced causal structure (same tables as the baseline):
# query slots bg=0..3 hold global 256-row blocks g=2*bg+h. For kv tile S
# (0..15), valid query cols start at LO(S) = 512*(S//8) + LO128[S%8]*128,
# and MASKS[S%8] lists (query-128-block offset, mask tile) additions.
LO128 = [0, 0, 0, 1, 2, 2, 2, 3]
MASKS = [
    [(0, "m1d")],
    [(0, "m1f"), (1, "m1d")],
    [(0, "m2d"), (1, "m1f")],
    [(1, "m2d")],
    [(2, "m1d")],
    [(2, "m1f"), (3, "m1d")],
    [(2, "m2d"), (3, "m1f")],
    [(3, "m2d")],
]


def lo_of(S):
    return 512 * (S // 8) + LO128[S % 8] * P


class _TC(tile.TileContext):
    """TileContext whose tail drain puts its global-clock waits on a nop
    (walrus rejects multi-wait Drain); excess waits are split by
    _split_waits() afterwards."""

    def _drain_and_barrier(self, tick_clock, wait_clock):
        nop_inst = self.nc.sync.nop(nofuse=True, hint="pre_drain_waits")
        wait_clock.add_sem_waits(
            nop_inst.ins, ScopedClock({None: tick_clock.global_clock})
        )
        self.nc.sync.drain()
        self.nc.all_engine_barrier()
        assert self.sems is not None
        popped = self.nc._tile_sem_poison_stack.pop()
        assert popped is self._sem_poison
        self.nc.clear_and_free_semaphores(list(self.sems.allocated().values()))
        self.nc.all_engine_barrier()


def _split_waits(nc, max_waits=_MAX_WAITS):
    """Walrus rejects instructions carrying more than `max_waits` sync waits.
    Move excess waits onto injected nops placed immediately before the
    instruction on the same engine (identical semantics)."""
    import copy

    template = nc.sync.nop(nofuse=True, hint="waitsplit_template").ins
    counter = [0]

    def make_nop(engine, waits):
        nop = copy.deepcopy(template)
        counter[0] += 1
        nop.name = f"I-wsplit-{counter[0]}"
        nop.engine = engine
        nop.sync_info = mybir.SyncInfo(on_wait=list(waits), on_update=[])
        return nop

    f = nc.m.functions[0]
    for bb in f.blocks:
        insts = bb.instructions
        if not any(
            i.sync_info and i.sync_info.on_wait and len(i.sync_info.on_wait) > max_waits
            for i in insts
        ):
            continue
        newlist = []
        for inst in insts:
            si = inst.sync_info
            if si and si.on_wait and len(si.on_wait) > max_waits:
                if inst.name == template.name:
                    newlist.append(inst)
                    continue
                waits = list(si.on_wait)
                del si.on_wait[max_waits:]
                rest = waits[max_waits:]
                while rest:
                    newlist.append(make_nop(inst.engine, rest[:max_waits]))
                    rest = rest[max_waits:]
            newlist.append(inst)
        bb.instructions[:] = newlist


def _chunks(lo, hi, step=512):
    out = []
    while lo < hi:
        w = min(step, hi - lo)
        out.append((lo, lo + w))
        lo += w
    return out


def _pair(tl, off, stride, w):
    """[128, 2, w] AP over tile `tl` starting at column `off`, middle-dim
    stride `stride` (elements) — a DoubleRow operand covering two
    128-contraction slices."""
    a = tl[:]
    pstr, pcnt = a.ap[0]
    return RAP(a.tensor, a.offset + off, [[pstr, pcnt], [stride, 2], [1, w]])


def _build_nc():
    nc = bass.Bass("TRN2", target_bir_lowering=False, debug=False)

    # DRAM I/O.  x layouts: [p, ct*2048 + t] (hi | lo halves); xq likewise
    # with the core's gathered interleaved query rows.  Weights [p, hi|lo of
    # ot*1024 + ct*128 + o] (k/q, stationary layout) or [p, ct*1024 + ch]
    # (v-folded, moving layout), pre-scaled x32.
    xd = nc.dram_tensor("xd", [P, 2 * NT * T], F8, kind="ExternalInput").ap()
    xqd = nc.dram_tensor("xqd", [P, 2 * NT * H], F8, kind="ExternalInput").ap()
    wkd = nc.dram_tensor("wkd", [P, 2 * NT * C], F8, kind="ExternalInput").ap()
    wqd = nc.dram_tensor("wqd", [P, 2 * NT * C], F8, kind="ExternalInput").ap()
    wvd = nc.dram_tensor("wvd", [P, 2 * NT * C], F8, kind="ExternalInput").ap()
    bkd = nc.dram_tensor("bkd", [P, NT], FP, kind="ExternalInput").ap()
    bqd = nc.dram_tensor("bqd", [P, NT], FP, kind="ExternalInput").ap()
    bed = nc.dram_tensor("bed", [P, NT], FP, kind="ExternalInput").ap()
    onesd = nc.dram_tensor("onesd", [P, 2 * P], F8, kind="ExternalInput").ap()
    ones16d = nc.dram_tensor("ones16d", [P, 2 * P], F8, kind="ExternalInput").ap()
    m1dd = nc.dram_tensor("m1dd", [P, P], FP, kind="ExternalInput").ap()
    m1fd = nc.dram_tensor("m1fd", [P, P], FP, kind="ExternalInput").ap()
    m2dd = nc.dram_tensor("m2dd", [P, P], FP, kind="ExternalInput").ap()
    ebd = nc.dram_tensor("ebd", [P, 2], FP, kind="ExternalInput").ap()
    # y out, bf16, tile-major: row block (ot*2 + chunk) holds [p, 512]
    yT = nc.dram_tensor("yT", [NT * 2 * P, 512], BF, kind="ExternalOutput").ap()

    with _TC(nc) as tc:
        with (
            tc.tile_pool(name="misc", bufs=1) as misc,
            tc.tile_pool(name="wpool", bufs=3) as wp,
            tc.tile_pool(name="kqv", bufs=1) as kqv,
            tc.tile_pool(name="scr", bufs=6) as scp,
            tc.tile_pool(name="yev", bufs=2) as yep,
            tc.tile_pool(name="psum", bufs=6, space="PSUM") as pp,
            tc.tile_pool(name="psum_rs", bufs=1, space="PSUM") as pp_rs,
        ):
            ones8 = misc.tile([P, 2 * P], F8, tag="ones")
            ones16 = misc.tile([P, 2 * P], F8, tag="ones16")
            m1d = misc.tile([P, P], FP, tag="m1d")
            m1f = misc.tile([P, P], FP, tag="m1f")
            m2d = misc.tile([P, P], FP, tag="m2d")
            bk_sb = misc.tile([P, NT], FP, tag="bk")
            bq_sb = misc.tile([P, NT], FP, tag="bq")
            be_sb = misc.tile([P, NT], FP, tag="be")
            rs_sb = misc.tile([P, H], FP, tag="rs")
            eb_sb = misc.tile([P, 2], FP, tag="eb")
            MT = {"m1d": m1d, "m1f": m1f, "m2d": m2d}

            kTh = kqv.tile([P, NT * T], F8, tag="kTh", name="kTh")
            kTl = kqv.tile([P, NT * T], F8, tag="kTl", name="kTl")
            qTh = kqv.tile([P, NT * H], F8, tag="qTh", name="qTh")
            qTl = kqv.tile([P, NT * H], F8, tag="qTl", name="qTl")
            vh = kqv.tile([P, TK * C], F8, tag="vh", name="vh")
            vl = kqv.tile([P, TK * C], F8, tag="vl", name="vl")
            v16 = kqv.tile([P, TK * C], F8, tag="v16", name="v16")

            def evac(ps, w, dsth, dstl, off, bias, scale, func=AF.Identity):
                """PSUM -> f32 scratch (Act) -> hi fp8 (Pool) -> lo fp8 (DVE)."""
                sc = scp.tile([P, 512], FP, tag="scr")
                nc.scalar.activation(sc[:, :w], ps[:, :w], func, bias=bias, scale=scale)
                nc.gpsimd.tensor_copy(dsth[:, off : off + w], sc[:, :w])
                nc.vector.tensor_sub(
                    dstl[:, off : off + w], sc[:, :w], dsth[:, off : off + w]
                )

            # 3-term DoubleRow accumulation helper.  terms = [(rhs_part_off,
            # lhs_part_off), ...] as (moving, stationary) hi/lo halves.
            TERMS = ((0, 0), (1, 0), (0, 1))

            # =========================================================
            # K projection: out [c(ot), kv] — lhsT = wk, rhs = x
            # =========================================================
            with tc.tile_pool(name="xp", bufs=1) as xp:
                xhl = xp.tile([P, 2 * NT * T], F8, tag="xhl", name="xhl")
                xq = xp.tile([P, 2 * NT * H], F8, tag="xq", name="xq")

                wk_h = wp.tile([P, NT * C], F8, tag="w", name="wk_h")
                wk_l = wp.tile([P, NT * C], F8, tag="w", name="wk_l")
                # critical-path DMA order: wk slice 0 (hi+lo), x chunk 0,
                # remaining wk slices, remaining x chunks, then the rest
                nc.sync.dma_start(wk_h[:, :C], wkd[:, :C])
                nc.sync.dma_start(wk_l[:, :C], wkd[:, NT * C : NT * C + C])

                def xchunk(part, c0):
                    dst = RAP(
                        xhl[:].tensor,
                        xhl[:].offset + part * NT * T + c0,
                        [[xhl[:].ap[0][0], P], [T, NT], [1, 512]],
                    )
                    src = RAP(
                        xd.tensor,
                        xd.offset + part * NT * T + c0,
                        [[xd.ap[0][0], P], [T, NT], [1, 512]],
                    )
                    nc.sync.dma_start(dst, src)

                xchunk(0, 0)
                xchunk(1, 0)
                nc.sync.dma_start(bk_sb[:], bkd)
                for ot in range(1, NT):
                    nc.sync.dma_start(wk_h[:, ot * C : ot * C + C],
                                      wkd[:, ot * C : ot * C + C])
                    nc.sync.dma_start(wk_l[:, ot * C : ot * C + C],
                                      wkd[:, (NT + ot) * C : (NT + ot) * C + C])
                for c in range(1, 4):
                    xchunk(0, c * 512)
                    xchunk(1, c * 512)
                nc.sync.dma_start(xq[:], xqd)
                wq_h = wp.tile([P, NT * C], F8, tag="w", name="wq_h")
                nc.sync.dma_start(wq_h[:], wqd[:, : NT * C])
                wq_l = wp.tile([P, NT * C], F8, tag="w", name="wq_l")
                nc.sync.dma_start(wq_l[:], wqd[:, NT * C :])
                nc.sync.dma_start(bq_sb[:], bqd)
                nc.sync.dma_start(eb_sb[:], ebd)

                sc_k = nc.named_scope("K")
                sc_k.__enter__()
                for cs, ce in _chunks(0, T):
                    for ot in range(NT):
                        w = ce - cs
                        ps = pp.tile([P, 512], FP, tag="ps", name=f"psk{ot}_{cs}")
                        n = 0
                        for rp, lp in TERMS:
                            for j in range(NT // 2):
                                nc.tensor.matmul(
                                    ps[:, :w],
                                    lhsT=_pair(
                                        wk_h if lp == 0 else wk_l,
                                        ot * C + j * 2 * P, P, P,
                                    ),
                                    rhs=_pair(
                                        xhl, rp * NT * T + j * 2 * T + cs, T, w
                                    ),
                                    start=(n == 0),
                                    stop=(n == 11),
                                    perf_mode=DR,
                                )
                                n += 1
                        evac(ps, w, kTh, kTl, ot * T + cs, bk_sb[:, ot : ot + 1], S32)
                sc_k.__exit__(None, None, None)

                sc_q = nc.named_scope("Q")
                sc_q.__enter__()
                for ot in range(NT):
                    for cs, ce in _chunks(0, H):
                        w = ce - cs
                        ps = pp.tile([P, 512], FP, tag="ps", name=f"psq{ot}_{cs}")
                        n = 0
                        for rp, lp in TERMS:
                            for j in range(NT // 2):
                                nc.tensor.matmul(
                                    ps[:, :w],
                                    lhsT=_pair(
                                        wq_h if lp == 0 else wq_l,
                                        ot * C + j * 2 * P, P, P,
                                    ),
                                    rhs=_pair(
                                        xq, rp * NT * H + j * 2 * H + cs, H, w
                                    ),
                                    start=(n == 0),
                                    stop=(n == 11),
                                    perf_mode=DR,
                                )
                                n += 1
                        evac(ps, w, qTh, qTl, ot * H + cs, bq_sb[:, ot : ot + 1], S32)
                sc_q.__exit__(None, None, None)

                wv_h = wp.tile([P, NT * C], F8, tag="w", name="wv_h")
                nc.sync.dma_start(wv_h[:], wvd[:, : NT * C])
                wv_l = wp.tile([P, NT * C], F8, tag="w", name="wv_l")
                nc.sync.dma_start(wv_l[:], wvd[:, NT * C :])
                nc.sync.dma_start(ones8[:], onesd)
                nc.sync.dma_start(ones16[:], ones16d)
                nc.sync.dma_start(m1d[:], m1dd)
                nc.sync.dma_start(m1f[:], m1fd)
                nc.sync.dma_start(m2d[:], m2dd)
                nc.sync.dma_start(be_sb[:], bed)

                sc_v = nc.named_scope("V")
                sc_v.__enter__()
                # folded-V projection: out [kv-rows(s), ch] — lhsT = x tile,
                # rhs = wv
                for s in range(TK):
                    for cs, ce in _chunks(0, C):
                        w = ce - cs
                        ps = pp.tile([P, 512], FP, tag="ps", name=f"psv{s}_{cs}")
                        n = 0
                        for rp, lp in TERMS:
                            for j in range(NT // 2):
                                nc.tensor.matmul(
                                    ps[:, :w],
                                    lhsT=_pair(
                                        xhl, lp * NT * T + j * 2 * T + s * P, T, P
                                    ),
                                    rhs=_pair(
                                        wv_h if rp == 0 else wv_l,
                                        j * 2 * C + cs, C, w,
                                    ),
                                    start=(n == 0),
                                    stop=(n == 11),
                                    perf_mode=DR,
                                )
                                n += 1
                        off = s * C + cs
                        sc = scp.tile([P, 512], FP, tag="scr")
                        nc.scalar.activation(
                            sc[:, :w], ps[:, :w], AF.Identity,
                            bias=eb_sb[:, 1:2], scale=S32,
                        )
                        nc.gpsimd.tensor_copy(vh[:, off : off + w], sc[:, :w])
                        nc.vector.tensor_sub(
                            vl[:, off : off + w], sc[:, :w], vh[:, off : off + w]
                        )
                        if (s + cs // 512) % 2 == 0:
                            nc.scalar.activation(
                                v16[:, off : off + w], sc[:, :w], AF.Identity,
                                bias=eb_sb[:, 1:2], scale=1.0 / 16.0,
                            )
                        else:
                            nc.gpsimd.tensor_scalar_mul(
                                v16[:, off : off + w], sc[:, :w], 1.0 / 16.0
                            )
                sc_v.__exit__(None, None, None)

            # =========================================================
            # Attention: x pool freed, A tensors reuse its space
            # =========================================================
            with tc.tile_pool(name="ap", bufs=1) as apool:
                Ah = apool.tile([P, TK * H], F8, tag="Ah", name="Ah")
                Al16 = apool.tile([P, TK * H], F8, tag="Al16", name="Al16")
                rs_ps = pp_rs.tile([P, H], FP, tag="rsps")

                # zero the pair-union gap regions (read by rowsum/attv,
                # never written by scores): tiles S=3,7,11,15
                for S in (3, 7, 11, 15):
                    g0 = lo_of(S - 1)
                    g1 = lo_of(S)
                    nc.gpsimd.memset(Ah[:, S * H + g0 : S * H + g1], 0.0)
                    nc.gpsimd.memset(Al16[:, S * H + g0 : S * H + g1], 0.0)

                sc_s = nc.named_scope("S")
                sc_s.__enter__()
                for S in range(TK):
                    base = 512 * (S // 8)
                    for cs, ce in _chunks(lo_of(S), H):
                        w = ce - cs
                        ps = pp.tile([P, 512], FP, tag="ps", name=f"pss{S}_{cs}")
                        n = 0
                        for rp, lp in TERMS:
                            kt = kTh if lp == 0 else kTl
                            qt = qTh if rp == 0 else qTl
                            for j in range(NT // 2):
                                nc.tensor.matmul(
                                    ps[:, :w],
                                    lhsT=_pair(kt, j * 2 * T + S * P, T, P),
                                    rhs=_pair(qt, j * 2 * H + cs, H, w),
                                    start=(n == 0),
                                    stop=(n == 11),
                                    perf_mode=DR,
                                )
                                n += 1
                        for moff, mname in MASKS[S % 8]:
                            a = base + moff * P
                            if cs <= a < ce:
                                nc.vector.tensor_add(
                                    ps[:, a - cs : a - cs + P],
                                    ps[:, a - cs : a - cs + P],
                                    MT[mname][:],
                                )
                        off = S * H + cs
                        sc = scp.tile([P, 512], FP, tag="scr")
                        nc.scalar.activation(
                            sc[:, :w], ps[:, :w], AF.Exp,
                            bias=eb_sb[:, 0:1], scale=S32,
                        )
                        nc.gpsimd.tensor_copy(Ah[:, off : off + w], sc[:, :w])
                        r32 = scp.tile([P, 512], FP, tag="scr")
                        nc.vector.tensor_sub(
                            r32[:, :w], sc[:, :w], Ah[:, off : off + w]
                        )
                        if (S + cs // 512) % 2 == 0:
                            nc.scalar.activation(
                                Al16[:, off : off + w], r32[:, :w], AF.Identity,
                                bias=eb_sb[:, 1:2], scale=16.0,
                            )
                        else:
                            nc.gpsimd.tensor_scalar_mul(
                                Al16[:, off : off + w], r32[:, :w], 16.0
                            )
                sc_s.__exit__(None, None, None)

                sc_r = nc.named_scope("R")
                sc_r.__enter__()
                # rowsums: ones @ (Ah | Al), DR pairs over kv tiles
                first = True
                for part, At in ((0, Ah), (1, Al16)):
                    ow = ones8 if part == 0 else ones16
                    for m in range(TK // 2):
                        lo = lo_of(2 * m)
                        for cs, ce in _chunks(lo, H):
                            w = ce - cs
                            nc.tensor.matmul(
                                rs_ps[:, cs:ce],
                                lhsT=_pair(ow, 0, P, P),
                                rhs=_pair(At, m * 2 * H + cs, H, w),
                                start=first and lo == 0,
                                stop=(part == 1 and m == TK // 2 - 1 and ce == H),
                                perf_mode=DR,
                            )
                        if lo == 0:
                            first = False
                nc.vector.reciprocal(rs_sb[:], rs_ps[:])
                sc_r.__exit__(None, None, None)

                sc_o = nc.named_scope("O")
                sc_o.__enter__()
                # attv: out [ch(ot), q] — lhsT = v, rhs = A; single
                # accumulation chain over all 16 kv tiles
                for ot in range(NT):
                    for cs, ce in _chunks(0, H):
                        ps = pp.tile([P, 512], FP, tag="ps", name=f"pso{ot}_{cs}")
                        mms = []
                        for At, vt in ((Ah, vh), (Al16, v16), (Ah, vl)):
                            for m in range(TK // 2):
                                lo = max(cs, lo_of(2 * m))
                                if lo >= ce:
                                    continue
                                mms.append(
                                    (
                                        _pair(vt, m * 2 * C + ot * P, C, P),
                                        _pair(At, m * 2 * H + lo, H, ce - lo),
                                        lo - cs,
                                        ce - lo,
                                    )
                                )
                        # widest range first so start=True covers everything
                        mms.sort(key=lambda t: t[3], reverse=True)
                        for i, (lt, rt, o0, w) in enumerate(mms):
                            nc.tensor.matmul(
                                ps[:, o0 : o0 + w],
                                lhsT=lt,
                                rhs=rt,
                                start=(i == 0),
                                stop=(i == len(mms) - 1),
                                perf_mode=DR,
                            )
                        w = ce - cs
                        ym = scp.tile([P, 512], FP, tag="scr")
                        nc.vector.tensor_mul(ym[:, :w], ps[:, :w], rs_sb[:, cs:ce])
                        ye = yep.tile([P, 512], BF, tag="ye")
                        nc.scalar.activation(
                            ye[:, :w], ym[:, :w], AF.Identity,
                            bias=be_sb[:, ot : ot + 1],
                        )
                        ci = cs // 512
                        nc.sync.dma_start(
                            yT[(ot * 2 + ci) * P : (ot * 2 + ci + 1) * P, :w],
                            ye[:, :w],
                        )
                sc_o.__exit__(None, None, None)

    _split_waits(nc)
    return nc


_NC_CACHE = None


def _get_nc():
    global _NC_CACHE
    if _NC_CACHE is None:
        _NC_CACHE = _build_nc()
    return _NC_CACHE


def _split8(a):
    """v -> (e4m3(v), e4m3(v - e4m3(v))) as fp8 arrays."""
    hi = a.astype(E4)
    lo = (a - hi.astype(np.float32)).astype(E4)
    return hi, lo


def _hl(a):
    h, l = _split8(np.ascontiguousarray(a, dtype=np.float32))
    return np.concatenate([h, l], axis=-1)


def make_in_maps(x, w_qkv, b_qkv, w_proj, b_proj):
    x = np.asarray(x, dtype=np.float32)
    w_qkv = np.asarray(w_qkv, dtype=np.float32)
    b_qkv = np.asarray(b_qkv, dtype=np.float32)
    w_proj = np.asarray(w_proj, dtype=np.float32)
    b_proj = np.asarray(b_proj, dtype=np.float32)

    wq, wk, wv = w_qkv[:C], w_qkv[C : 2 * C], w_qkv[2 * C :]
    bq, bk, bv = b_qkv[:C], b_qkv[C : 2 * C], b_qkv[2 * C :]
    wt = w_proj @ wv                       # folded V*proj weight
    beff = b_proj + w_proj @ bv

    def pack_stat(w):
        # [p, ot*1024 + ct*128 + o] = 32*w[ot*128+o, ct*128+p]
        w4 = (32.0 * w).reshape(NT, P, NT, P)       # [ot, o, ct, p]
        return w4.transpose(3, 0, 2, 1).reshape(P, NT * C)

    def pack_mov(w):
        # [p, ct*1024 + ch] = 32*w[ch, ct*128+p]
        w3 = (32.0 * w).reshape(C, NT, P)           # [ch, ct, p]
        return w3.transpose(2, 1, 0).reshape(P, NT * C)

    def pack_x(xr):
        # [p, ct*Tr + t] = xr[t, ct*128+p]
        Tr = xr.shape[0]
        x3 = xr.T.reshape(NT, P, Tr)                # [ct, p, t]
        return x3.transpose(1, 0, 2).reshape(P, NT * Tr)

    wkp = _hl(pack_stat(wk))
    wqp = _hl(pack_stat(wq))
    wvp = _hl(pack_mov(wt))
    bkp = np.ascontiguousarray(bk.reshape(NT, P).T)
    bqp = np.ascontiguousarray(bq.reshape(NT, P).T)
    bep = np.ascontiguousarray(beff.reshape(NT, P).T)

    ones = np.ones((P, 2 * P), dtype=np.float32).astype(E4)
    ones16 = np.full((P, 2 * P), 1.0 / 16.0, dtype=np.float32).astype(E4)
    triu = np.triu(np.ones((P, P), dtype=np.float32))
    trilm = np.where(triu > 0, 0.0, NEG).astype(np.float32)
    zeros = np.zeros((P, P), dtype=np.float32)
    negs = np.full((P, P), NEG, dtype=np.float32)

    shared = dict(
        wkd=wkp, wqd=wqp, wvd=wvp, bkd=bkp, bqd=bqp, bed=bep, onesd=ones,
        ones16d=ones16,
        ebd=np.concatenate(
            [np.full((P, 1), ELN32, np.float32), np.zeros((P, 1), np.float32)],
            axis=1,
        ),
    )
    in_maps = []
    for core in range(8):
        b, h = core // 2, core % 2
        xb = x[b]
        qrows = np.concatenate(
            [xb[(2 * bg + h) * 256 : (2 * bg + h + 1) * 256] for bg in range(4)],
            axis=0,
        )
        in_maps.append(
            dict(
                shared,
                xd=_hl(pack_x(xb)),
                xqd=_hl(pack_x(qrows)),
                m1dd=trilm if h == 0 else zeros,
                m1fd=negs if h == 0 else zeros,
                m2dd=negs if h == 0 else trilm,
            )
        )
    return in_maps


def assemble_output(results):
    B = 4
    y = np.empty((B, T, C), dtype=np.float32)
    for core in range(8):
        b, h = core // 2, core % 2
        yt = np.asarray(results[core]["yT"], dtype=np.float32)
        yt = yt.reshape(NT, 2, P, 512)
        full = yt.transpose(1, 3, 0, 2).reshape(H, C)   # [q-col, ch]
        for bg in range(4):
            g = 2 * bg + h
            y[b, g * 256 : (g + 1) * 256, :] = full[bg * 256 : (bg + 1) * 256]
    return y


def kernel(x, w_qkv, b_qkv, w_proj, b_proj):
    from concourse.bass_utils import run_bass_kernel_spmd

    nc = _get_nc()
    in_maps = make_in_maps(x, w_qkv, b_qkv, w_proj, b_proj)
    res = run_bass_kernel_spmd(nc, in_maps, list(range(8)))
    return assemble_output(res.results)


# revision 12
# speedup vs baseline: 1.5546x; 1.0788x over previous
"""Single-head causal attention (B=4, T=2048, C=1024) on 8 trn2 NeuronCores.

Sharding: 8 shards = (batch b in 0..3) x (query interleave h in 0..1); core h
takes interleaved 256-row query blocks {2*bg+h}, balancing the causal
triangle. One SPMD stream; per-core variation is data only (gathered q-rows
and three [128,128] mask tiles).

Math restructure vs the old baseline:
  - W_proj is folded into W_v host-side (Wt = w_proj @ w_v), deleting the
    output-projection phase entirely: y = (A @ (x Wt^T)) / rowsum + beff.
  - All matmuls run as fp8e4 DoubleRow (0.5 cycles/row, 256-deep contraction
    per instruction) with 3-term hi/lo error compensation: each operand v is
    split v = vh + vl (vh = e4m3(v), vl = e4m3(v - vh), both at natural
    scale) and products use vh*wh + vl*wh + vh*wl (the lo*lo term is ~eps^2
    and dropped). Effective precision ~bf16 at 0.75x the PE cost of bf16.
    Splits of x and the weights are free (host-side); k/q/V/A splits ride
    the existing PSUM-evacuation passes (Act: f32 scratch, Pool: hi cast,
    DVE: lo = scratch - hi).
  - Weights are shipped x32 (fp8-friendly range); the 1/32 un-scale rides
    the evacuation activations; 1/sqrt(C) rides the exp activation scale;
    exp carries a -ln(32) bias for fp8 headroom (cancels in softmax).
  - Everything stays in SBUF (fp8 halves footprints): no DRAM spill, no
    phase C reload, single attv accumulation chain over all 16 kv tiles.
"""

import sys

sys.path.insert(0, "/opt/trn_rl_repo")

import numpy as np
import ml_dtypes

import concourse.bass as bass
import concourse.tile as tile
from concourse import mybir
from concourse.vector_clock import ScopedClock
from bass_rust import AP as RAP

FP = mybir.dt.float32
BF = mybir.dt.bfloat16
F8 = mybir.dt.float8e4
AF = mybir.ActivationFunctionType
DR = mybir.MatmulPerfMode.DoubleRow
E4 = ml_dtypes.float8_e4m3

P = 128
C = 1024           # embed dim
NT = C // P        # 8 contraction tiles
T = 2048           # kv length per core
TK = T // P        # 16 kv tiles
H = 1024           # query cols per core
NEG = -1.0e9
ELN32 = -3.4657359  # -ln(32): exp headroom bias, cancels in softmax
S32 = 1.0 / 32.0

_MAX_WAITS = 1

# Interleaved-256 balanced causal structure (same tables as the baseline):
# query slots bg=0..3 hold global 256-row blocks g=2*bg+h. For kv tile S
# (0..15), valid query cols start at LO(S) = 512*(S//8) + LO128[S%8]*128,
# and MASKS[S%8] lists (query-128-block offset, mask tile) additions.
LO128 = [0, 0, 0, 1, 2, 2, 2, 3]
MASKS = [
    [(0, "m1d")],
    [(0, "m1f"), (1, "m1d")],
    [(0, "m2d"), (1, "m1f")],
    [(1, "m2d")],
    [(2, "m1d")],
    [(2, "m1f"), (3, "m1d")],
    [(2, "m2d"), (3, "m1f")],
    [(3, "m2d")],
]


def lo_of(S):
    return 512 * (S // 8) + LO128[S % 8] * P


class _TC(tile.TileContext):
    """TileContext whose tail drain puts its global-clock waits on a nop
    (walrus rejects multi-wait Drain); excess waits are split by
    _split_waits() afterwards."""

    def _drain_and_barrier(self, tick_clock, wait_clock):
        nop_inst = self.nc.sync.nop(nofuse=True, hint="pre_drain_waits")
        wait_clock.add_sem_waits(
            nop_inst.ins, ScopedClock({None: tick_clock.global_clock})
        )
        self.nc.sync.drain()
        self.nc.all_engine_barrier()
        assert self.sems is not None
        popped = self.nc._tile_sem_poison_stack.pop()
        assert popped is self._sem_poison
        self.nc.clear_and_free_semaphores(list(self.sems.allocated().values()))
        self.nc.all_engine_barrier()


def _split_waits(nc, max_waits=_MAX_WAITS):
    """Walrus rejects instructions carrying more than `max_waits` sync waits.
    Move excess waits onto injected nops placed immediately before the
    instruction on the same engine (identical semantics)."""
    import copy

    template = nc.sync.nop(nofuse=True, hint="waitsplit_template").ins
    counter = [0]

    def make_nop(engine, waits):
        nop = copy.deepcopy(template)
        counter[0] += 1
        nop.name = f"I-wsplit-{counter[0]}"
        nop.engine = engine
        nop.sync_info = mybir.SyncInfo(on_wait=list(waits), on_update=[])
        return nop

    f = nc.m.functions[0]
    for bb in f.blocks:
        insts = bb.instructions
        if not any(
            i.sync_info and i.sync_info.on_wait and len(i.sync_info.on_wait) > max_waits
            for i in insts
        ):
            continue
        newlist = []
        for inst in insts:
            si = inst.sync_info
            if si and si.on_wait and len(si.on_wait) > max_waits:
                if inst.name == template.name:
                    newlist.append(inst)
                    continue
                waits = list(si.on_wait)
                del si.on_wait[max_waits:]
                rest = waits[max_waits:]
                while rest:
                    newlist.append(make_nop(inst.engine, rest[:max_waits]))
                    rest = rest[max_waits:]
            newlist.append(inst)
        bb.instructions[:] = newlist


def _chunks(lo, hi, step=512):
    out = []
    while lo < hi:
        w = min(step, hi - lo)
        out.append((lo, lo + w))
        lo += w
    return out


def _pair(tl, off, stride, w):
    """[128, 2, w] AP over tile `tl` starting at column `off`, middle-dim
    stride `stride` (elements) — a DoubleRow operand covering two
    128-contraction slices."""
    a = tl[:]
    pstr, pcnt = a.ap[0]
    return RAP(a.tensor, a.offset + off, [[pstr, pcnt], [stride, 2], [1, w]])


def _build_nc():
    nc = bass.Bass("TRN2", target_bir_lowering=False, debug=False)

    # DRAM I/O.  x layouts: [p, ct*2048 + t] (hi | lo halves); xq likewise
    # with the core's gathered interleaved query rows.  Weights [p, hi|lo of
    # ot*1024 + ct*128 + o] (k/q, stationary layout) or [p, ct*1024 + ch]
    # (v-folded, moving layout), pre-scaled x32.
    xd = nc.dram_tensor("xd", [P, 2 * NT * T], F8, kind="ExternalInput").ap()
    xqd = nc.dram_tensor("xqd", [P, 2 * NT * H], F8, kind="ExternalInput").ap()
    wkd = nc.dram_tensor("wkd", [P, 2 * NT * C], F8, kind="ExternalInput").ap()
    wqd = nc.dram_tensor("wqd", [P, 2 * NT * C], F8, kind="ExternalInput").ap()
    wvd = nc.dram_tensor("wvd", [P, 2 * NT * C], F8, kind="ExternalInput").ap()
    bkd = nc.dram_tensor("bkd", [P, NT], FP, kind="ExternalInput").ap()
    bqd = nc.dram_tensor("bqd", [P, NT], FP, kind="ExternalInput").ap()
    bed = nc.dram_tensor("bed", [P, NT], FP, kind="ExternalInput").ap()
    onesd = nc.dram_tensor("onesd", [P, 2 * P], F8, kind="ExternalInput").ap()
    ones16d = nc.dram_tensor("ones16d", [P, 2 * P], F8, kind="ExternalInput").ap()
    m1dd = nc.dram_tensor("m1dd", [P, 2 * P], F8, kind="ExternalInput").ap()
    m1fd = nc.dram_tensor("m1fd", [P, 2 * P], F8, kind="ExternalInput").ap()
    m2dd = nc.dram_tensor("m2dd", [P, 2 * P], F8, kind="ExternalInput").ap()
    identd = nc.dram_tensor("identd", [P, 2 * P], F8, kind="ExternalInput").ap()
    ebd = nc.dram_tensor("ebd", [P, 2], FP, kind="ExternalInput").ap()
    # y out, bf16, tile-major: row block (ot*2 + chunk) holds [p, 512]
    yT = nc.dram_tensor("yT", [NT * 2 * P, 512], BF, kind="ExternalOutput").ap()

    with _TC(nc) as tc:
        with (
            tc.tile_pool(name="misc", bufs=1) as misc,
            tc.tile_pool(name="wpool", bufs=3) as wp,
            tc.tile_pool(name="kqv", bufs=1) as kqv,
            tc.tile_pool(name="scr", bufs=5) as scp,
            tc.tile_pool(name="yev", bufs=4) as yep,
            tc.tile_pool(name="psum", bufs=6, space="PSUM") as pp,
            tc.tile_pool(name="psum_rs", bufs=1, space="PSUM") as pp_rs,
        ):
            ones8 = misc.tile([P, 2 * P], F8, tag="ones")
            ones16 = misc.tile([P, 2 * P], F8, tag="ones16")
            m1d = misc.tile([P, 2 * P], F8, tag="m1d")
            m1f = misc.tile([P, 2 * P], F8, tag="m1f")
            m2d = misc.tile([P, 2 * P], F8, tag="m2d")
            ident8 = misc.tile([P, 2 * P], F8, tag="ident")
            bk_sb = misc.tile([P, NT], FP, tag="bk")
            bq_sb = misc.tile([P, NT], FP, tag="bq")
            be_sb = misc.tile([P, NT], FP, tag="be")
            rs_sb = misc.tile([P, H], FP, tag="rs")
            eb_sb = misc.tile([P, 2], FP, tag="eb")
            MT = {"m1d": m1d, "m1f": m1f, "m2d": m2d}

            kTh = kqv.tile([P, NT * T], F8, tag="kTh", name="kTh")
            kTl = kqv.tile([P, NT * T], F8, tag="kTl", name="kTl")
            qTh = kqv.tile([P, NT * H], F8, tag="qTh", name="qTh")
            qTl = kqv.tile([P, NT * H], F8, tag="qTl", name="qTl")
            vh = kqv.tile([P, TK * C], F8, tag="vh", name="vh")
            vl = kqv.tile([P, TK * C], F8, tag="vl", name="vl")
            v16 = kqv.tile([P, TK * C], F8, tag="v16", name="v16")

            def evac(ps, w, dsth, dstl, off, bias, scale, func=AF.Identity):
                """PSUM -> f32 scratch (Act) -> hi fp8 (Pool) -> lo fp8 (DVE)."""
                sc = scp.tile([P, 512], FP, tag="scr")
                nc.scalar.activation(sc[:, :w], ps[:, :w], func, bias=bias, scale=scale)
                nc.gpsimd.tensor_copy(dsth[:, off : off + w], sc[:, :w])
                nc.vector.tensor_sub(
                    dstl[:, off : off + w], sc[:, :w], dsth[:, off : off + w]
                )

            # 3-term DoubleRow accumulation helper.  terms = [(rhs_part_off,
            # lhs_part_off), ...] as (moving, stationary) hi/lo halves.
            TERMS = ((0, 0), (1, 0), (0, 1))

            # =========================================================
            # K projection: out [c(ot), kv] — lhsT = wk, rhs = x
            # =========================================================
            with tc.tile_pool(name="xp", bufs=1) as xp:
                xhl = xp.tile([P, 2 * NT * T], F8, tag="xhl", name="xhl")
                xq = xp.tile([P, 2 * NT * H], F8, tag="xq", name="xq")

                wk_h = wp.tile([P, NT * C], F8, tag="w", name="wk_h")
                wk_l = wp.tile([P, NT * C], F8, tag="w", name="wk_l")
                # critical-path DMA order: wk slice 0 (hi+lo), x chunk 0,
                # remaining wk slices, remaining x chunks, then the rest
                nc.sync.dma_start(wk_h[:, :C], wkd[:, :C])
                nc.sync.dma_start(wk_l[:, :C], wkd[:, NT * C : NT * C + C])

                def xchunk(part, c0):
                    dst = RAP(
                        xhl[:].tensor,
                        xhl[:].offset + part * NT * T + c0,
                        [[xhl[:].ap[0][0], P], [T, NT], [1, 512]],
                    )
                    src = RAP(
                        xd.tensor,
                        xd.offset + part * NT * T + c0,
                        [[xd.ap[0][0], P], [T, NT], [1, 512]],
                    )
                    nc.sync.dma_start(dst, src)

                xchunk(0, 0)
                xchunk(1, 0)
                nc.sync.dma_start(bk_sb[:], bkd)
                for ot in range(1, NT):
                    nc.sync.dma_start(wk_h[:, ot * C : ot * C + C],
                                      wkd[:, ot * C : ot * C + C])
                    nc.sync.dma_start(wk_l[:, ot * C : ot * C + C],
                                      wkd[:, (NT + ot) * C : (NT + ot) * C + C])
                for c in range(1, 4):
                    xchunk(0, c * 512)
                    xchunk(1, c * 512)
                nc.sync.dma_start(xq[:], xqd)
                wq_h = wp.tile([P, NT * C], F8, tag="w", name="wq_h")
                nc.sync.dma_start(wq_h[:], wqd[:, : NT * C])
                wq_l = wp.tile([P, NT * C], F8, tag="w", name="wq_l")
                nc.sync.dma_start(wq_l[:], wqd[:, NT * C :])
                nc.sync.dma_start(bq_sb[:], bqd)
                nc.sync.dma_start(eb_sb[:], ebd)

                sc_k = nc.named_scope("K")
                sc_k.__enter__()
                for cs, ce in _chunks(0, T):
                    for ot in range(NT):
                        w = ce - cs
                        ps = pp.tile([P, 512], FP, tag="ps", name=f"psk{ot}_{cs}")
                        n = 0
                        for rp, lp in TERMS:
                            for j in range(NT // 2):
                                nc.tensor.matmul(
                                    ps[:, :w],
                                    lhsT=_pair(
                                        wk_h if lp == 0 else wk_l,
                                        ot * C + j * 2 * P, P, P,
                                    ),
                                    rhs=_pair(
                                        xhl, rp * NT * T + j * 2 * T + cs, T, w
                                    ),
                                    start=(n == 0),
                                    stop=(n == 11),
                                    perf_mode=DR,
                                )
                                n += 1
                        evac(ps, w, kTh, kTl, ot * T + cs, bk_sb[:, ot : ot + 1], S32)
                sc_k.__exit__(None, None, None)

                sc_q = nc.named_scope("Q")
                sc_q.__enter__()
                for ot in range(NT):
                    for cs, ce in _chunks(0, H):
                        w = ce - cs
                        ps = pp.tile([P, 512], FP, tag="ps", name=f"psq{ot}_{cs}")
                        n = 0
                        for rp, lp in TERMS:
                            for j in range(NT // 2):
                                nc.tensor.matmul(
                                    ps[:, :w],
                                    lhsT=_pair(
                                        wq_h if lp == 0 else wq_l,
                                        ot * C + j * 2 * P, P, P,
                                    ),
                                    rhs=_pair(
                                        xq, rp * NT * H + j * 2 * H + cs, H, w
                                    ),
                                    start=(n == 0),
                                    stop=(n == 11),
                                    perf_mode=DR,
                                )
                                n += 1
                        evac(ps, w, qTh, qTl, ot * H + cs, bq_sb[:, ot : ot + 1], S32)
                sc_q.__exit__(None, None, None)

                wv_h = wp.tile([P, NT * C], F8, tag="w", name="wv_h")
                nc.sync.dma_start(wv_h[:], wvd[:, : NT * C])
                wv_l = wp.tile([P, NT * C], F8, tag="w", name="wv_l")
                nc.sync.dma_start(wv_l[:], wvd[:, NT * C :])
                nc.sync.dma_start(ones8[:], onesd)
                nc.sync.dma_start(ones16[:], ones16d)
                nc.sync.dma_start(m1d[:], m1dd)
                nc.sync.dma_start(m1f[:], m1fd)
                nc.sync.dma_start(m2d[:], m2dd)
                nc.sync.dma_start(ident8[:], identd)
                nc.sync.dma_start(be_sb[:], bed)

                sc_v = nc.named_scope("V")
                sc_v.__enter__()
                # folded-V projection: out [kv-rows(s), ch] — lhsT = x tile,
                # rhs = wv
                for s in range(TK):
                    for cs, ce in _chunks(0, C):
                        w = ce - cs
                        ps = pp.tile([P, 512], FP, tag="ps", name=f"psv{s}_{cs}")
                        n = 0
                        for rp, lp in TERMS:
                            for j in range(NT // 2):
                                nc.tensor.matmul(
                                    ps[:, :w],
                                    lhsT=_pair(
                                        xhl, lp * NT * T + j * 2 * T + s * P, T, P
                                    ),
                                    rhs=_pair(
                                        wv_h if rp == 0 else wv_l,
                                        j * 2 * C + cs, C, w,
                                    ),
                                    start=(n == 0),
                                    stop=(n == 11),
                                    perf_mode=DR,
                                )
                                n += 1
                        off = s * C + cs
                        sc = scp.tile([P, 512], FP, tag="scr")
                        nc.scalar.activation(
                            sc[:, :w], ps[:, :w], AF.Identity,
                            bias=eb_sb[:, 1:2], scale=S32,
                        )
                        nc.gpsimd.tensor_copy(vh[:, off : off + w], sc[:, :w])
                        nc.vector.tensor_sub(
                            vl[:, off : off + w], sc[:, :w], vh[:, off : off + w]
                        )
                        nc.vector.tensor_scalar_mul(
                            v16[:, off : off + w], sc[:, :w], 1.0 / 16.0
                        )
                sc_v.__exit__(None, None, None)

            # =========================================================
            # Attention: x pool freed, A tensors reuse its space
            # =========================================================
            with tc.tile_pool(name="ap", bufs=1) as apool:
                Ah = apool.tile([P, TK * H], F8, tag="Ah", name="Ah")
                Al16 = apool.tile([P, TK * H], F8, tag="Al16", name="Al16")
                rs_ps = pp_rs.tile([P, H], FP, tag="rsps")

                # zero the pair-union gap regions (read by rowsum/attv,
                # never written by scores): tiles S=3,7,11,15
                for S in (3, 7, 11, 15):
                    g0 = lo_of(S - 1)
                    g1 = lo_of(S)
                    nc.gpsimd.memset(Ah[:, S * H + g0 : S * H + g1], 0.0)
                    nc.gpsimd.memset(Al16[:, S * H + g0 : S * H + g1], 0.0)

                sc_s = nc.named_scope("S")
                sc_s.__enter__()
                for S in range(TK):
                    base = 512 * (S // 8)
                    for cs, ce in _chunks(lo_of(S), H):
                        w = ce - cs
                        ps = pp.tile([P, 512], FP, tag="ps", name=f"pss{S}_{cs}")
                        # 12 score matmuls + mask matmuls (mask^T @ I adds the
                        # causal -448 pattern inside the same psum chain)
                        mm = []
                        for rp, lp in TERMS:
                            kt = kTh if lp == 0 else kTl
                            qt = qTh if rp == 0 else qTl
                            for j in range(NT // 2):
                                mm.append((
                                    ps[:, :w],
                                    _pair(kt, j * 2 * T + S * P, T, P),
                                    _pair(qt, j * 2 * H + cs, H, w),
                                ))
                        for moff, mname in MASKS[S % 8]:
                            a = base + moff * P
                            if cs <= a < ce:
                                mm.insert(len(mm) - 1, (
                                    ps[:, a - cs : a - cs + P],
                                    _pair(MT[mname], 0, P, P),
                                    _pair(ident8, 0, P, P),
                                ))
                        for n, (po, lt, rt) in enumerate(mm):
                            nc.tensor.matmul(
                                po, lhsT=lt, rhs=rt,
                                start=(n == 0), stop=(n == len(mm) - 1),
                                perf_mode=DR,
                            )
                        off = S * H + cs
                        sc = scp.tile([P, 512], FP, tag="scr")
                        nc.scalar.activation(
                            sc[:, :w], ps[:, :w], AF.Exp,
                            bias=eb_sb[:, 0:1], scale=S32,
                        )
                        nc.gpsimd.tensor_copy(Ah[:, off : off + w], sc[:, :w])
                        r32 = scp.tile([P, 512], FP, tag="scr")
                        nc.vector.tensor_sub(
                            r32[:, :w], sc[:, :w], Ah[:, off : off + w]
                        )
                        nc.vector.tensor_scalar_mul(
                            Al16[:, off : off + w], r32[:, :w], 16.0
                        )
                sc_s.__exit__(None, None, None)

                sc_r = nc.named_scope("R")
                sc_r.__enter__()
                # rowsums: ones @ (Ah | Al), DR pairs over kv tiles
                first = True
                for part, At in ((0, Ah), (1, Al16)):
                    ow = ones8 if part == 0 else ones16
                    for m in range(TK // 2):
                        lo = lo_of(2 * m)
                        for cs, ce in _chunks(lo, H):
                            w = ce - cs
                            nc.tensor.matmul(
                                rs_ps[:, cs:ce],
                                lhsT=_pair(ow, 0, P, P),
                                rhs=_pair(At, m * 2 * H + cs, H, w),
                                start=first and lo == 0,
                                stop=(part == 1 and m == TK // 2 - 1 and ce == H),
                                perf_mode=DR,
                            )
                        if lo == 0:
                            first = False
                nc.vector.reciprocal(rs_sb[:], rs_ps[:])
                sc_r.__exit__(None, None, None)

                sc_o = nc.named_scope("O")
                sc_o.__enter__()
                # attv: out [ch(ot), q] — lhsT = v, rhs = A; single
                # accumulation chain over all 16 kv tiles
                for ot in range(NT):
                    for cs, ce in _chunks(0, H):
                        ps = pp.tile([P, 512], FP, tag="ps", name=f"pso{ot}_{cs}")
                        mms = []
                        for At, vt in ((Ah, vh), (Al16, v16), (Ah, vl)):
                            for m in range(TK // 2):
                                lo = max(cs, lo_of(2 * m))
                                if lo >= ce:
                                    continue
                                mms.append(
                                    (
                                        _pair(vt, m * 2 * C + ot * P, C, P),
                                        _pair(At, m * 2 * H + lo, H, ce - lo),
                                        lo - cs,
                                        ce - lo,
                                    )
                                )
                        # widest range first so start=True covers everything
                        mms.sort(key=lambda t: t[3], reverse=True)
                        for i, (lt, rt, o0, w) in enumerate(mms):
                            nc.tensor.matmul(
                                ps[:, o0 : o0 + w],
                                lhsT=lt,
                                rhs=rt,
                                start=(i == 0),
                                stop=(i == len(mms) - 1),
                                perf_mode=DR,
                            )
                        w = ce - cs
                        ym = scp.tile([P, 512], FP, tag="scr")
                        nc.vector.tensor_mul(ym[:, :w], ps[:, :w], rs_sb[:, cs:ce])
                        ye = yep.tile([P, 512], BF, tag="ye")
                        nc.scalar.activation(
                            ye[:, :w], ym[:, :w], AF.Identity,
                            bias=be_sb[:, ot : ot + 1],
                        )
                        ci = cs // 512
                        nc.sync.dma_start(
                            yT[(ot * 2 + ci) * P : (ot * 2 + ci + 1) * P, :w],
                            ye[:, :w],
                        )
                sc_o.__exit__(None, None, None)

    _split_waits(nc)
    return nc


_NC_CACHE = None


def _get_nc():
    global _NC_CACHE
    if _NC_CACHE is None:
        _NC_CACHE = _build_nc()
    return _NC_CACHE


def _split8(a):
    """v -> (e4m3(v), e4m3(v - e4m3(v))) as fp8 arrays."""
    hi = a.astype(E4)
    lo = (a - hi.astype(np.float32)).astype(E4)
    return hi, lo


def _hl(a):
    h, l = _split8(np.ascontiguousarray(a, dtype=np.float32))
    return np.concatenate([h, l], axis=-1)


def make_in_maps(x, w_qkv, b_qkv, w_proj, b_proj):
    x = np.asarray(x, dtype=np.float32)
    w_qkv = np.asarray(w_qkv, dtype=np.float32)
    b_qkv = np.asarray(b_qkv, dtype=np.float32)
    w_proj = np.asarray(w_proj, dtype=np.float32)
    b_proj = np.asarray(b_proj, dtype=np.float32)

    wq, wk, wv = w_qkv[:C], w_qkv[C : 2 * C], w_qkv[2 * C :]
    bq, bk, bv = b_qkv[:C], b_qkv[C : 2 * C], b_qkv[2 * C :]
    wt = w_proj @ wv                       # folded V*proj weight
    beff = b_proj + w_proj @ bv

    def pack_stat(w):
        # [p, ot*1024 + ct*128 + o] = 32*w[ot*128+o, ct*128+p]
        w4 = (32.0 * w).reshape(NT, P, NT, P)       # [ot, o, ct, p]
        return w4.transpose(3, 0, 2, 1).reshape(P, NT * C)

    def pack_mov(w):
        # [p, ct*1024 + ch] = 32*w[ch, ct*128+p]
        w3 = (32.0 * w).reshape(C, NT, P)           # [ch, ct, p]
        return w3.transpose(2, 1, 0).reshape(P, NT * C)

    def pack_x(xr):
        # [p, ct*Tr + t] = xr[t, ct*128+p]
        Tr = xr.shape[0]
        x3 = xr.T.reshape(NT, P, Tr)                # [ct, p, t]
        return x3.transpose(1, 0, 2).reshape(P, NT * Tr)

    wkp = _hl(pack_stat(wk))
    wqp = _hl(pack_stat(wq))
    wvp = _hl(pack_mov(wt))
    bkp = np.ascontiguousarray(bk.reshape(NT, P).T)
    bqp = np.ascontiguousarray(bq.reshape(NT, P).T)
    bep = np.ascontiguousarray(beff.reshape(NT, P).T)

    ones = np.ones((P, 2 * P), dtype=np.float32).astype(E4)
    ones16 = np.full((P, 2 * P), 1.0 / 16.0, dtype=np.float32).astype(E4)
    # mask tiles ship TRANSPOSED (lhsT of mask^T @ I), duplicated [m|m]
    # so both DoubleRow slices add the pattern: effective bias 2*(-240)
    M8 = -240.0
    tril = np.tril(np.ones((P, P), dtype=np.float32))
    trilmT = np.where(tril > 0, 0.0, M8).astype(np.float32)
    trilmT = np.concatenate([trilmT, trilmT], axis=1).astype(E4)
    zeros = np.zeros((P, 2 * P), dtype=np.float32).astype(E4)
    negs = np.full((P, 2 * P), M8, dtype=np.float32).astype(E4)
    ident = np.concatenate(
        [np.eye(P, dtype=np.float32), np.eye(P, dtype=np.float32)], axis=1
    ).astype(E4)

    shared = dict(
        wkd=wkp, wqd=wqp, wvd=wvp, bkd=bkp, bqd=bqp, bed=bep, onesd=ones,
        ones16d=ones16, identd=ident,
        ebd=np.concatenate(
            [np.full((P, 1), ELN32, np.float32), np.zeros((P, 1), np.float32)],
            axis=1,
        ),
    )
    in_maps = []
    for core in range(8):
        b, h = core // 2, core % 2
        xb = x[b]
        qrows = np.concatenate(
            [xb[(2 * bg + h) * 256 : (2 * bg + h + 1) * 256] for bg in range(4)],
            axis=0,
        )
        in_maps.append(
            dict(
                shared,
                xd=_hl(pack_x(xb)),
                xqd=_hl(pack_x(qrows)),
                m1dd=trilmT if h == 0 else zeros,
                m1fd=negs if h == 0 else zeros,
                m2dd=negs if h == 0 else trilmT,
            )
        )
    return in_maps


def assemble_output(results):
    B = 4
    y = np.empty((B, T, C), dtype=np.float32)
    for core in range(8):
        b, h = core // 2, core % 2
        yt = np.asarray(results[core]["yT"], dtype=np.float32)
        yt = yt.reshape(NT, 2, P, 512)
        full = yt.transpose(1, 3, 0, 2).reshape(H, C)   # [q-col, ch]
        for bg in range(4):
            g = 2 * bg + h
            y[b, g * 256 : (g + 1) * 256, :] = full[bg * 256 : (bg + 1) * 256]
    return y


def kernel(x, w_qkv, b_qkv, w_proj, b_proj):
    from concourse.bass_utils import run_bass_kernel_spmd

    nc = _get_nc()
    in_maps = make_in_maps(x, w_qkv, b_qkv, w_proj, b_proj)
    res = run_bass_kernel_spmd(nc, in_maps, list(range(8)))
    return assemble_output(res.results)


# revision 13
# speedup vs baseline: 1.5585x; 1.0025x over previous
"""Single-head causal attention (B=4, T=2048, C=1024) on 8 trn2 NeuronCores.

Sharding: 8 shards = (batch b in 0..3) x (query interleave h in 0..1); core h
takes interleaved 256-row query blocks {2*bg+h}, balancing the causal
triangle. One SPMD stream; per-core variation is data only (gathered q-rows
and three [128,128] mask tiles).

Math restructure vs the old baseline:
  - W_proj is folded into W_v host-side (Wt = w_proj @ w_v), deleting the
    output-projection phase entirely: y = (A @ (x Wt^T)) / rowsum + beff.
  - All matmuls run as fp8e4 DoubleRow (0.5 cycles/row, 256-deep contraction
    per instruction) with 3-term hi/lo error compensation: each operand v is
    split v = vh + vl (vh = e4m3(v), vl = e4m3(v - vh), both at natural
    scale) and products use vh*wh + vl*wh + vh*wl (the lo*lo term is ~eps^2
    and dropped). Effective precision ~bf16 at 0.75x the PE cost of bf16.
    Splits of x and the weights are free (host-side); k/q/V/A splits ride
    the existing PSUM-evacuation passes (Act: f32 scratch, Pool: hi cast,
    DVE: lo = scratch - hi).
  - Weights are shipped x32 (fp8-friendly range); the 1/32 un-scale rides
    the evacuation activations; 1/sqrt(C) rides the exp activation scale;
    exp carries a -ln(32) bias for fp8 headroom (cancels in softmax).
  - Everything stays in SBUF (fp8 halves footprints): no DRAM spill, no
    phase C reload, single attv accumulation chain over all 16 kv tiles.
"""

import sys

sys.path.insert(0, "/opt/trn_rl_repo")

import numpy as np
import ml_dtypes

import concourse.bass as bass
import concourse.tile as tile
from concourse import mybir
from concourse.vector_clock import ScopedClock
from bass_rust import AP as RAP

FP = mybir.dt.float32
BF = mybir.dt.bfloat16
F8 = mybir.dt.float8e4
AF = mybir.ActivationFunctionType
DR = mybir.MatmulPerfMode.DoubleRow
E4 = ml_dtypes.float8_e4m3

P = 128
C = 1024           # embed dim
NT = C // P        # 8 contraction tiles
T = 2048           # kv length per core
TK = T // P        # 16 kv tiles
H = 1024           # query cols per core
NEG = -1.0e9
ELN32 = -3.4657359  # -ln(32): exp headroom bias, cancels in softmax
S32 = 1.0 / 32.0

_MAX_WAITS = 1

# Interleaved-256 balanced causal structure (same tables as the baseline):
# query slots bg=0..3 hold global 256-row blocks g=2*bg+h. For kv tile S
# (0..15), valid query cols start at LO(S) = 512*(S//8) + LO128[S%8]*128,
# and MASKS[S%8] lists (query-128-block offset, mask tile) additions.
LO128 = [0, 0, 0, 1, 2, 2, 2, 3]
MASKS = [
    [(0, "m1d")],
    [(0, "m1f"), (1, "m1d")],
    [(0, "m2d"), (1, "m1f")],
    [(1, "m2d")],
    [(2, "m1d")],
    [(2, "m1f"), (3, "m1d")],
    [(2, "m2d"), (3, "m1f")],
    [(3, "m2d")],
]


def lo_of(S):
    return 512 * (S // 8) + LO128[S % 8] * P


class _TC(tile.TileContext):
    """TileContext whose tail drain puts its global-clock waits on a nop
    (walrus rejects multi-wait Drain); excess waits are split by
    _split_waits() afterwards."""

    def _drain_and_barrier(self, tick_clock, wait_clock):
        nop_inst = self.nc.sync.nop(nofuse=True, hint="pre_drain_waits")
        wait_clock.add_sem_waits(
            nop_inst.ins, ScopedClock({None: tick_clock.global_clock})
        )
        self.nc.sync.drain()
        self.nc.all_engine_barrier()
        assert self.sems is not None
        popped = self.nc._tile_sem_poison_stack.pop()
        assert popped is self._sem_poison
        self.nc.clear_and_free_semaphores(list(self.sems.allocated().values()))
        self.nc.all_engine_barrier()


def _split_waits(nc, max_waits=_MAX_WAITS):
    """Walrus rejects instructions carrying more than `max_waits` sync waits.
    Move excess waits onto injected nops placed immediately before the
    instruction on the same engine (identical semantics)."""
    import copy

    template = nc.sync.nop(nofuse=True, hint="waitsplit_template").ins
    counter = [0]

    def make_nop(engine, waits):
        nop = copy.deepcopy(template)
        counter[0] += 1
        nop.name = f"I-wsplit-{counter[0]}"
        nop.engine = engine
        nop.sync_info = mybir.SyncInfo(on_wait=list(waits), on_update=[])
        return nop

    f = nc.m.functions[0]
    for bb in f.blocks:
        insts = bb.instructions
        if not any(
            i.sync_info and i.sync_info.on_wait and len(i.sync_info.on_wait) > max_waits
            for i in insts
        ):
            continue
        newlist = []
        for inst in insts:
            si = inst.sync_info
            if si and si.on_wait and len(si.on_wait) > max_waits:
                if inst.name == template.name:
                    newlist.append(inst)
                    continue
                waits = list(si.on_wait)
                del si.on_wait[max_waits:]
                rest = waits[max_waits:]
                while rest:
                    newlist.append(make_nop(inst.engine, rest[:max_waits]))
                    rest = rest[max_waits:]
            newlist.append(inst)
        bb.instructions[:] = newlist


def _chunks(lo, hi, step=512):
    out = []
    while lo < hi:
        w = min(step, hi - lo)
        out.append((lo, lo + w))
        lo += w
    return out


def _pair(tl, off, stride, w):
    """[128, 2, w] AP over tile `tl` starting at column `off`, middle-dim
    stride `stride` (elements) — a DoubleRow operand covering two
    128-contraction slices."""
    a = tl[:]
    pstr, pcnt = a.ap[0]
    return RAP(a.tensor, a.offset + off, [[pstr, pcnt], [stride, 2], [1, w]])


def _build_nc():
    nc = bass.Bass("TRN2", target_bir_lowering=False, debug=False)

    # DRAM I/O.  x layouts: [p, ct*2048 + t] (hi | lo halves); xq likewise
    # with the core's gathered interleaved query rows.  Weights [p, hi|lo of
    # ot*1024 + ct*128 + o] (k/q, stationary layout) or [p, ct*1024 + ch]
    # (v-folded, moving layout), pre-scaled x32.
    xd = nc.dram_tensor("xd", [P, 2 * NT * T], F8, kind="ExternalInput").ap()
    xqd = nc.dram_tensor("xqd", [P, 2 * NT * H], F8, kind="ExternalInput").ap()
    wkd = nc.dram_tensor("wkd", [P, 2 * NT * C], F8, kind="ExternalInput").ap()
    wqd = nc.dram_tensor("wqd", [P, 2 * NT * C], F8, kind="ExternalInput").ap()
    wvd = nc.dram_tensor("wvd", [P, 2 * NT * C], F8, kind="ExternalInput").ap()
    bkd = nc.dram_tensor("bkd", [P, NT], FP, kind="ExternalInput").ap()
    bqd = nc.dram_tensor("bqd", [P, NT], FP, kind="ExternalInput").ap()
    bed = nc.dram_tensor("bed", [P, NT], FP, kind="ExternalInput").ap()
    onesd = nc.dram_tensor("onesd", [P, 2 * P], F8, kind="ExternalInput").ap()
    ones16d = nc.dram_tensor("ones16d", [P, 2 * P], F8, kind="ExternalInput").ap()
    m1dd = nc.dram_tensor("m1dd", [P, 2 * P], F8, kind="ExternalInput").ap()
    m1fd = nc.dram_tensor("m1fd", [P, 2 * P], F8, kind="ExternalInput").ap()
    m2dd = nc.dram_tensor("m2dd", [P, 2 * P], F8, kind="ExternalInput").ap()
    identd = nc.dram_tensor("identd", [P, 2 * P], F8, kind="ExternalInput").ap()
    ebd = nc.dram_tensor("ebd", [P, 2], FP, kind="ExternalInput").ap()
    # y out, bf16, tile-major: row block (ot*2 + chunk) holds [p, 512]
    yT = nc.dram_tensor("yT", [NT * 2 * P, 512], BF, kind="ExternalOutput").ap()

    with _TC(nc) as tc:
        with (
            tc.tile_pool(name="misc", bufs=1) as misc,
            tc.tile_pool(name="wpool", bufs=3) as wp,
            tc.tile_pool(name="kqv", bufs=1) as kqv,
            tc.tile_pool(name="scr", bufs=5) as scp,
            tc.tile_pool(name="yev", bufs=4) as yep,
            tc.tile_pool(name="psum", bufs=6, space="PSUM") as pp,
            tc.tile_pool(name="psum_rs", bufs=1, space="PSUM") as pp_rs,
        ):
            ones8 = misc.tile([P, 2 * P], F8, tag="ones")
            ones16 = misc.tile([P, 2 * P], F8, tag="ones16")
            m1d = misc.tile([P, 2 * P], F8, tag="m1d")
            m1f = misc.tile([P, 2 * P], F8, tag="m1f")
            m2d = misc.tile([P, 2 * P], F8, tag="m2d")
            ident8 = misc.tile([P, 2 * P], F8, tag="ident")
            bk_sb = misc.tile([P, NT], FP, tag="bk")
            bq_sb = misc.tile([P, NT], FP, tag="bq")
            be_sb = misc.tile([P, NT], FP, tag="be")
            rs_sb = misc.tile([P, H], FP, tag="rs")
            eb_sb = misc.tile([P, 2], FP, tag="eb")
            MT = {"m1d": m1d, "m1f": m1f, "m2d": m2d}

            kTh = kqv.tile([P, NT * T], F8, tag="kTh", name="kTh")
            kTl = kqv.tile([P, NT * T], F8, tag="kTl", name="kTl")
            qTh = kqv.tile([P, NT * H], F8, tag="qTh", name="qTh")
            qTl = kqv.tile([P, NT * H], F8, tag="qTl", name="qTl")
            vh = kqv.tile([P, TK * C], F8, tag="vh", name="vh")
            vl = kqv.tile([P, TK * C], F8, tag="vl", name="vl")
            v16 = kqv.tile([P, TK * C], F8, tag="v16", name="v16")

            def evac(ps, w, dsth, dstl, off, bias, scale, func=AF.Identity):
                """PSUM -> f32 scratch (Act) -> hi fp8 (Pool) -> lo fp8 (DVE)."""
                sc = scp.tile([P, 512], FP, tag="scr")
                nc.scalar.activation(sc[:, :w], ps[:, :w], func, bias=bias, scale=scale)
                nc.gpsimd.tensor_copy(dsth[:, off : off + w], sc[:, :w])
                nc.vector.tensor_sub(
                    dstl[:, off : off + w], sc[:, :w], dsth[:, off : off + w]
                )

            # 3-term DoubleRow accumulation helper.  terms = [(rhs_part_off,
            # lhs_part_off), ...] as (moving, stationary) hi/lo halves.
            TERMS = ((0, 0), (0, 1), (1, 0))

            # =========================================================
            # K projection: out [c(ot), kv] — lhsT = wk, rhs = x
            # =========================================================
            with tc.tile_pool(name="xp", bufs=1) as xp:
                xhl = xp.tile([P, 2 * NT * T], F8, tag="xhl", name="xhl")
                xq = xp.tile([P, 2 * NT * H], F8, tag="xq", name="xq")

                wk_h = wp.tile([P, NT * C], F8, tag="w", name="wk_h")
                wk_l = wp.tile([P, NT * C], F8, tag="w", name="wk_l")
                # critical-path DMA order: wk slice 0 (hi+lo), x chunk 0,
                # remaining wk slices, remaining x chunks, then the rest
                nc.sync.dma_start(wk_h[:, :C], wkd[:, :C])
                nc.sync.dma_start(wk_l[:, :C], wkd[:, NT * C : NT * C + C])

                def xchunk(part, c0):
                    dst = RAP(
                        xhl[:].tensor,
                        xhl[:].offset + part * NT * T + c0,
                        [[xhl[:].ap[0][0], P], [T, NT], [1, 512]],
                    )
                    src = RAP(
                        xd.tensor,
                        xd.offset + part * NT * T + c0,
                        [[xd.ap[0][0], P], [T, NT], [1, 512]],
                    )
                    nc.sync.dma_start(dst, src)

                xchunk(0, 0)
                xchunk(1, 0)
                nc.sync.dma_start(bk_sb[:], bkd)
                for ot in range(1, NT):
                    nc.sync.dma_start(wk_h[:, ot * C : ot * C + C],
                                      wkd[:, ot * C : ot * C + C])
                    nc.sync.dma_start(wk_l[:, ot * C : ot * C + C],
                                      wkd[:, (NT + ot) * C : (NT + ot) * C + C])
                for c in range(1, 4):
                    xchunk(0, c * 512)
                    xchunk(1, c * 512)
                nc.sync.dma_start(xq[:], xqd)
                wq_h = wp.tile([P, NT * C], F8, tag="w", name="wq_h")
                nc.sync.dma_start(wq_h[:], wqd[:, : NT * C])
                wq_l = wp.tile([P, NT * C], F8, tag="w", name="wq_l")
                nc.sync.dma_start(wq_l[:], wqd[:, NT * C :])
                nc.sync.dma_start(bq_sb[:], bqd)
                nc.sync.dma_start(eb_sb[:], ebd)

                sc_k = nc.named_scope("K")
                sc_k.__enter__()
                for cs, ce in _chunks(0, T):
                    for ot in range(NT):
                        w = ce - cs
                        ps = pp.tile([P, 512], FP, tag="ps", name=f"psk{ot}_{cs}")
                        n = 0
                        for rp, lp in TERMS:
                            for j in range(NT // 2):
                                nc.tensor.matmul(
                                    ps[:, :w],
                                    lhsT=_pair(
                                        wk_h if lp == 0 else wk_l,
                                        ot * C + j * 2 * P, P, P,
                                    ),
                                    rhs=_pair(
                                        xhl, rp * NT * T + j * 2 * T + cs, T, w
                                    ),
                                    start=(n == 0),
                                    stop=(n == 11),
                                    perf_mode=DR,
                                )
                                n += 1
                        evac(ps, w, kTh, kTl, ot * T + cs, bk_sb[:, ot : ot + 1], S32)
                sc_k.__exit__(None, None, None)

                sc_q = nc.named_scope("Q")
                sc_q.__enter__()
                for ot in range(NT):
                    for cs, ce in _chunks(0, H):
                        w = ce - cs
                        ps = pp.tile([P, 512], FP, tag="ps", name=f"psq{ot}_{cs}")
                        n = 0
                        for rp, lp in TERMS:
                            for j in range(NT // 2):
                                nc.tensor.matmul(
                                    ps[:, :w],
                                    lhsT=_pair(
                                        wq_h if lp == 0 else wq_l,
                                        ot * C + j * 2 * P, P, P,
                                    ),
                                    rhs=_pair(
                                        xq, rp * NT * H + j * 2 * H + cs, H, w
                                    ),
                                    start=(n == 0),
                                    stop=(n == 11),
                                    perf_mode=DR,
                                )
                                n += 1
                        evac(ps, w, qTh, qTl, ot * H + cs, bq_sb[:, ot : ot + 1], S32)
                sc_q.__exit__(None, None, None)

                wv_h = wp.tile([P, NT * C], F8, tag="w", name="wv_h")
                nc.sync.dma_start(wv_h[:], wvd[:, : NT * C])
                wv_l = wp.tile([P, NT * C], F8, tag="w", name="wv_l")
                nc.sync.dma_start(wv_l[:], wvd[:, NT * C :])
                nc.sync.dma_start(ones8[:], onesd)
                nc.sync.dma_start(ones16[:], ones16d)
                nc.sync.dma_start(m1d[:], m1dd)
                nc.sync.dma_start(m1f[:], m1fd)
                nc.sync.dma_start(m2d[:], m2dd)
                nc.sync.dma_start(ident8[:], identd)
                nc.sync.dma_start(be_sb[:], bed)

                sc_v = nc.named_scope("V")
                sc_v.__enter__()
                # folded-V projection: out [kv-rows(s), ch] — lhsT = x tile,
                # rhs = wv
                for s in range(TK):
                    for cs, ce in _chunks(0, C):
                        w = ce - cs
                        ps = pp.tile([P, 512], FP, tag="ps", name=f"psv{s}_{cs}")
                        n = 0
                        for rp, lp in TERMS:
                            for j in range(NT // 2):
                                nc.tensor.matmul(
                                    ps[:, :w],
                                    lhsT=_pair(
                                        xhl, lp * NT * T + j * 2 * T + s * P, T, P
                                    ),
                                    rhs=_pair(
                                        wv_h if rp == 0 else wv_l,
                                        j * 2 * C + cs, C, w,
                                    ),
                                    start=(n == 0),
                                    stop=(n == 11),
                                    perf_mode=DR,
                                )
                                n += 1
                        off = s * C + cs
                        sc = scp.tile([P, 512], FP, tag="scr")
                        nc.scalar.activation(
                            sc[:, :w], ps[:, :w], AF.Identity,
                            bias=eb_sb[:, 1:2], scale=S32,
                        )
                        nc.gpsimd.tensor_copy(vh[:, off : off + w], sc[:, :w])
                        nc.vector.tensor_sub(
                            vl[:, off : off + w], sc[:, :w], vh[:, off : off + w]
                        )
                        nc.vector.tensor_scalar_mul(
                            v16[:, off : off + w], sc[:, :w], 1.0 / 16.0
                        )
                sc_v.__exit__(None, None, None)

            # =========================================================
            # Attention: x pool freed, A tensors reuse its space
            # =========================================================
            with tc.tile_pool(name="ap", bufs=1) as apool:
                Ah = apool.tile([P, TK * H], F8, tag="Ah", name="Ah")
                Al16 = apool.tile([P, TK * H], F8, tag="Al16", name="Al16")
                rs_ps = pp_rs.tile([P, H], FP, tag="rsps")

                # zero the pair-union gap regions (read by rowsum/attv,
                # never written by scores): tiles S=3,7,11,15
                for S in (3, 7, 11, 15):
                    g0 = lo_of(S - 1)
                    g1 = lo_of(S)
                    nc.gpsimd.memset(Ah[:, S * H + g0 : S * H + g1], 0.0)
                    nc.gpsimd.memset(Al16[:, S * H + g0 : S * H + g1], 0.0)

                sc_s = nc.named_scope("S")
                sc_s.__enter__()
                for S in range(TK):
                    base = 512 * (S // 8)
                    for cs, ce in _chunks(lo_of(S), H):
                        w = ce - cs
                        ps = pp.tile([P, 512], FP, tag="ps", name=f"pss{S}_{cs}")
                        # 12 score matmuls + mask matmuls (mask^T @ I adds the
                        # causal -448 pattern inside the same psum chain)
                        mm = []
                        for rp, lp in TERMS:
                            kt = kTh if lp == 0 else kTl
                            qt = qTh if rp == 0 else qTl
                            for j in range(NT // 2):
                                mm.append((
                                    ps[:, :w],
                                    _pair(kt, j * 2 * T + S * P, T, P),
                                    _pair(qt, j * 2 * H + cs, H, w),
                                ))
                        for moff, mname in MASKS[S % 8]:
                            a = base + moff * P
                            if cs <= a < ce:
                                mm.insert(len(mm) - 1, (
                                    ps[:, a - cs : a - cs + P],
                                    _pair(MT[mname], 0, P, P),
                                    _pair(ident8, 0, P, P),
                                ))
                        for n, (po, lt, rt) in enumerate(mm):
                            nc.tensor.matmul(
                                po, lhsT=lt, rhs=rt,
                                start=(n == 0), stop=(n == len(mm) - 1),
                                perf_mode=DR,
                            )
                        off = S * H + cs
                        sc = scp.tile([P, 512], FP, tag="scr")
                        nc.scalar.activation(
                            sc[:, :w], ps[:, :w], AF.Exp,
                            bias=eb_sb[:, 0:1], scale=S32,
                        )
                        nc.gpsimd.tensor_copy(Ah[:, off : off + w], sc[:, :w])
                        r32 = scp.tile([P, 512], FP, tag="scr")
                        nc.vector.tensor_sub(
                            r32[:, :w], sc[:, :w], Ah[:, off : off + w]
                        )
                        nc.vector.tensor_scalar_mul(
                            Al16[:, off : off + w], r32[:, :w], 16.0
                        )
                sc_s.__exit__(None, None, None)

                sc_r = nc.named_scope("R")
                sc_r.__enter__()
                # rowsums: ones @ (Ah | Al), DR pairs over kv tiles
                first = True
                for part, At in ((0, Ah), (1, Al16)):
                    ow = ones8 if part == 0 else ones16
                    for m in range(TK // 2):
                        lo = lo_of(2 * m)
                        for cs, ce in _chunks(lo, H):
                            w = ce - cs
                            nc.tensor.matmul(
                                rs_ps[:, cs:ce],
                                lhsT=_pair(ow, 0, P, P),
                                rhs=_pair(At, m * 2 * H + cs, H, w),
                                start=first and lo == 0,
                                stop=(part == 1 and m == TK // 2 - 1 and ce == H),
                                perf_mode=DR,
                            )
                        if lo == 0:
                            first = False
                nc.vector.reciprocal(rs_sb[:], rs_ps[:])
                sc_r.__exit__(None, None, None)

                sc_o = nc.named_scope("O")
                sc_o.__enter__()
                # attv: out [ch(ot), q] — lhsT = v, rhs = A; single
                # accumulation chain over all 16 kv tiles
                for ot in range(NT):
                    for cs, ce in _chunks(0, H):
                        ps = pp.tile([P, 512], FP, tag="ps", name=f"pso{ot}_{cs}")
                        mms = []
                        for At, vt in ((Ah, vh), (Al16, v16), (Ah, vl)):
                            for m in range(TK // 2):
                                lo = max(cs, lo_of(2 * m))
                                if lo >= ce:
                                    continue
                                mms.append(
                                    (
                                        _pair(vt, m * 2 * C + ot * P, C, P),
                                        _pair(At, m * 2 * H + lo, H, ce - lo),
                                        lo - cs,
                                        ce - lo,
                                    )
                                )
                        # widest range first so start=True covers everything
                        mms.sort(key=lambda t: t[3], reverse=True)
                        for i, (lt, rt, o0, w) in enumerate(mms):
                            nc.tensor.matmul(
                                ps[:, o0 : o0 + w],
                                lhsT=lt,
                                rhs=rt,
                                start=(i == 0),
                                stop=(i == len(mms) - 1),
                                perf_mode=DR,
                            )
                        w = ce - cs
                        ym = scp.tile([P, 512], FP, tag="scr")
                        nc.vector.tensor_mul(ym[:, :w], ps[:, :w], rs_sb[:, cs:ce])
                        ye = yep.tile([P, 512], BF, tag="ye")
                        nc.scalar.activation(
                            ye[:, :w], ym[:, :w], AF.Identity,
                            bias=be_sb[:, ot : ot + 1],
                        )
                        ci = cs // 512
                        nc.sync.dma_start(
                            yT[(ot * 2 + ci) * P : (ot * 2 + ci + 1) * P, :w],
                            ye[:, :w],
                        )
                sc_o.__exit__(None, None, None)

    _split_waits(nc)
    return nc


_NC_CACHE = None


def _get_nc():
    global _NC_CACHE
    if _NC_CACHE is None:
        _NC_CACHE = _build_nc()
    return _NC_CACHE


def _split8(a):
    """v -> (e4m3(v), e4m3(v - e4m3(v))) as fp8 arrays."""
    hi = a.astype(E4)
    lo = (a - hi.astype(np.float32)).astype(E4)
    return hi, lo


def _hl(a):
    h, l = _split8(np.ascontiguousarray(a, dtype=np.float32))
    return np.concatenate([h, l], axis=-1)


def make_in_maps(x, w_qkv, b_qkv, w_proj, b_proj):
    x = np.asarray(x, dtype=np.float32)
    w_qkv = np.asarray(w_qkv, dtype=np.float32)
    b_qkv = np.asarray(b_qkv, dtype=np.float32)
    w_proj = np.asarray(w_proj, dtype=np.float32)
    b_proj = np.asarray(b_proj, dtype=np.float32)

    wq, wk, wv = w_qkv[:C], w_qkv[C : 2 * C], w_qkv[2 * C :]
    bq, bk, bv = b_qkv[:C], b_qkv[C : 2 * C], b_qkv[2 * C :]
    wt = w_proj @ wv                       # folded V*proj weight
    beff = b_proj + w_proj @ bv

    def pack_stat(w):
        # [p, ot*1024 + ct*128 + o] = 32*w[ot*128+o, ct*128+p]
        w4 = (32.0 * w).reshape(NT, P, NT, P)       # [ot, o, ct, p]
        return w4.transpose(3, 0, 2, 1).reshape(P, NT * C)

    def pack_mov(w):
        # [p, ct*1024 + ch] = 32*w[ch, ct*128+p]
        w3 = (32.0 * w).reshape(C, NT, P)           # [ch, ct, p]
        return w3.transpose(2, 1, 0).reshape(P, NT * C)

    def pack_x(xr):
        # [p, ct*Tr + t] = xr[t, ct*128+p]
        Tr = xr.shape[0]
        x3 = xr.T.reshape(NT, P, Tr)                # [ct, p, t]
        return x3.transpose(1, 0, 2).reshape(P, NT * Tr)

    wkp = _hl(pack_stat(wk))
    wqp = _hl(pack_stat(wq))
    wvp = _hl(pack_mov(wt))
    bkp = np.ascontiguousarray(bk.reshape(NT, P).T)
    bqp = np.ascontiguousarray(bq.reshape(NT, P).T)
    bep = np.ascontiguousarray(beff.reshape(NT, P).T)

    ones = np.ones((P, 2 * P), dtype=np.float32).astype(E4)
    ones16 = np.full((P, 2 * P), 1.0 / 16.0, dtype=np.float32).astype(E4)
    # mask tiles ship TRANSPOSED (lhsT of mask^T @ I), duplicated [m|m]
    # so both DoubleRow slices add the pattern: effective bias 2*(-240)
    M8 = -240.0
    tril = np.tril(np.ones((P, P), dtype=np.float32))
    trilmT = np.where(tril > 0, 0.0, M8).astype(np.float32)
    trilmT = np.concatenate([trilmT, trilmT], axis=1).astype(E4)
    zeros = np.zeros((P, 2 * P), dtype=np.float32).astype(E4)
    negs = np.full((P, 2 * P), M8, dtype=np.float32).astype(E4)
    ident = np.concatenate(
        [np.eye(P, dtype=np.float32), np.eye(P, dtype=np.float32)], axis=1
    ).astype(E4)

    shared = dict(
        wkd=wkp, wqd=wqp, wvd=wvp, bkd=bkp, bqd=bqp, bed=bep, onesd=ones,
        ones16d=ones16, identd=ident,
        ebd=np.concatenate(
            [np.full((P, 1), ELN32, np.float32), np.zeros((P, 1), np.float32)],
            axis=1,
        ),
    )
    in_maps = []
    for core in range(8):
        b, h = core // 2, core % 2
        xb = x[b]
        qrows = np.concatenate(
            [xb[(2 * bg + h) * 256 : (2 * bg + h + 1) * 256] for bg in range(4)],
            axis=0,
        )
        in_maps.append(
            dict(
                shared,
                xd=_hl(pack_x(xb)),
                xqd=_hl(pack_x(qrows)),
                m1dd=trilmT if h == 0 else zeros,
                m1fd=negs if h == 0 else zeros,
                m2dd=negs if h == 0 else trilmT,
            )
        )
    return in_maps


def assemble_output(results):
    B = 4
    y = np.empty((B, T, C), dtype=np.float32)
    for core in range(8):
        b, h = core // 2, core % 2
        yt = np.asarray(results[core]["yT"], dtype=np.float32)
        yt = yt.reshape(NT, 2, P, 512)
        full = yt.transpose(1, 3, 0, 2).reshape(H, C)   # [q-col, ch]
        for bg in range(4):
            g = 2 * bg + h
            y[b, g * 256 : (g + 1) * 256, :] = full[bg * 256 : (bg + 1) * 256]
    return y


def kernel(x, w_qkv, b_qkv, w_proj, b_proj):
    from concourse.bass_utils import run_bass_kernel_spmd

    nc = _get_nc()
    in_maps = make_in_maps(x, w_qkv, b_qkv, w_proj, b_proj)
    res = run_bass_kernel_spmd(nc, in_maps, list(range(8)))
    return assemble_output(res.results)
